# revision 40
# baseline (speedup 1.0000x reference)
"""Single-head attention (B=16, T=2048, C=576, H=96) on 8 TRN2 NeuronCores.

Sharding: data-parallel over batch — 2 batches per core; weights replicated.

All matmul operands are bf16 (fp32 PSUM accumulation); rel-err budget is
2e-2 and bf16 end-to-end measures ~7e-3. The host pre-transposes x to
xT=[C,T] bf16 and packs W=[5,128,288] (zero-padded C) bf16, so the device
never transposes anything:

  qT,kT [96,T]  = W-slice stationary, xT moving          (PSUM-accum over C)
  v_nat [T,96]  = xT-tile stationary, Wv-slice moving    (natural layout)
  vp [128,NT,98]: v*mask with mask in cols 96,97 (denominator trick)
  sT [kpos,qpos] per (qc, 2-kt group) into a 2-bank PSUM tile (bufs=2) so
     the next group's score matmuls overlap the previous group's exp
  out_nat [qpos,98] = es-slice stationary, vp moving, accumulated over kt;
     col 96 is the softmax denominator. DVE reciprocal+scale, DMA out with
     a (p j) permutation for 1536B descriptors; host un-permutes.

All x loads ride the SP/HWDGE queue in priority order (b0 half 0 first);
the tiny mask loads go through the idle GpSimd engine's SWDGE so they do
not consume SP issue slots.

This walrus build rejects >1 sync wait per instruction (and any wait on a
Drain), so after TileContext builds the module we hoist excess waits onto
injected same-engine NOPs — semantics identical since engines execute
their stream in order.
"""

import sys

if "/opt/trn_rl_repo" not in sys.path:
    sys.path.insert(0, "/opt/trn_rl_repo")

import ml_dtypes
import numpy as np

import concourse.bass as bass
import concourse.tile as tile
from concourse import mybir
from concourse.bass_utils import run_bass_kernel_spmd

N_CORES = 8
B, T, C, H = 16, 2048, 576, 96
BPC = B // N_CORES  # batches per core
SCALE = 1.0 / float(np.sqrt(H))

F32 = mybir.dt.float32
BF16 = mybir.dt.bfloat16

NT = T // 128  # 16 key tiles
NCT = (C + 127) // 128  # 5 c-tiles (last is 64)
NQC = T // 512  # 4 query chunks
NG = 8  # kt-groups per query chunk (2 kt each)
KG = NT // NG  # 2 kt per group
HP = H + 2  # 98: H + denominator col + dup (even moving count)
POOL_EXP_GROUPS = ()  # which kt-groups' exp runs on GpSimd instead of ACT


def _split_excess_waits(nc, max_waits=1):
    """Hoist sync waits beyond this walrus's per-instruction limit onto
    injected NOPs that run just before, on the same engine."""
    n_split = 0
    for fn in nc.m.functions:
        for blk in fn.blocks:
            new_insts = []
            changed = False
            for inst in blk.instructions:
                si = inst.sync_info
                waits = list(si.on_wait) if si is not None else []
                cap = 0 if isinstance(inst, mybir.InstDrain) else max_waits
                if len(waits) > cap:
                    # Keep the most meaningful wait ON the instruction (its
                    # engine-stage wait doesn't block the sequencer); push
                    # self-engine sems (trivially satisfied in-order) and
                    # DMA-completion WARs onto the NOPs, which DO block SEQ.
                    eng = str(inst.engine).split(".")[-1].split(":")[0].strip("'\" >")

                    def prio(iw):
                        i, w = iw
                        nm = getattr(w, "ant_name", "") or ""
                        self_sem = nm.startswith(eng)
                        dma_sem = nm.startswith("DMAHW") or nm.startswith("DMASW")
                        return (0 if self_sem else (1 if dma_sem else 2), i)

                    order = sorted(enumerate(waits), key=prio)
                    waits = [w for _, w in order]
                    excess = waits[:-cap] if cap else waits
                    keep = waits[-cap:] if cap else []
                    for i in range(0, len(excess), max_waits):
                        chunk = excess[i : i + max_waits]
                        new_insts.append(
                            mybir.InstNoOp(
                                name=f"{inst.name}-wsplit{i}",
                                engine=inst.engine,
                                ins=[],
                                outs=[],
                                sync_info=mybir.SyncInfo(on_wait=chunk, on_update=[]),
                            )
                        )
                    inst.sync_info = mybir.SyncInfo(
                        on_wait=keep, on_update=list(si.on_update)
                    )
                    changed = True
                    n_split += 1
                new_insts.append(inst)
            if changed:
                blk.instructions = new_insts
    return n_split


def _build():
    nc = bass.Bass("TRN2", target_bir_lowering=False, debug=False)

    xt_d = nc.dram_tensor("xt", [BPC, C, T], BF16, kind="ExternalInput")
    w_d = nc.dram_tensor("w", [NCT * 128, 3 * H], BF16, kind="ExternalInput")
    mf_d = nc.dram_tensor("maskf", [BPC, 128, NT], F32, kind="ExternalInput")
    out_d = nc.dram_tensor("out", [BPC, T, H], F32, kind="ExternalOutput")

    exp = mybir.ActivationFunctionType.Exp

    with tile.TileContext(nc) as tc:
        with (
            tc.tile_pool(name="const", bufs=1) as const_pool,
            tc.tile_pool(name="xt", bufs=2) as xt_pool,
            tc.tile_pool(name="qk", bufs=2) as qk_pool,
            tc.tile_pool(name="vp", bufs=2) as vp_pool,
            tc.tile_pool(name="mk", bufs=2) as mk_pool,
            tc.tile_pool(name="es", bufs=17) as es_pool,
            tc.tile_pool(name="ot", bufs=4) as ot_pool,
            tc.tile_pool(name="psp", bufs=2, space="PSUM") as psp,  # proj+v shared
            tc.tile_pool(name="pss", bufs=2, space="PSUM") as pss,  # scores 2-bank
            tc.tile_pool(name="pso", bufs=2, space="PSUM") as pso,  # out chains
        ):
            # PE p-state warm-up: dependency-free dummy matmuls on garbage
            # SBUF data while the head DMAs land (PE is idle anyway). After
            # ~3us of continuous busy the cost model (and hardware) runs the
            # array at full clock, so the first real matmuls aren't 2x slow.
            # The psum bank is reset by its next user's start=True matmul.
            dmy = const_pool.tile([128, 640], BF16, name="dmy")
            nc.vector.memset(dmy[:], 0)
            dpo = pso.tile([128, 512], F32, tag="o", name="dpo")
            for _ in range(1):
                nc.tensor.matmul(
                    dpo[:, :],
                    dmy[:, 0:128],
                    dmy[:, 128:640],
                    start=True,
                    stop=True,
                )

            w_sb = const_pool.tile([128, NCT, 3 * H], BF16, name="w_sb")
            nc.sync.dma_start(
                w_sb[:], w_d.ap().rearrange("(g p) c -> p g c", p=128)
            )
            # pre-warm the exp table so the first real exp doesn't pay the
            # table load inside the pipeline
            warm = const_pool.tile([128, 2], F32, name="warm")
            nc.scalar.activation(warm[:], w_sb[:, 0, 0:4].bitcast(F32), exp)

            state = {}

            def mk_mask(b):
                def go():
                    mf = mk_pool.tile([128, NT], F32, name=f"mf{b}")
                    state[b]["mf"] = mf
                    nc.gpsimd.dma_start(mf[:], mf_d.ap()[b])

                return go

            def mk_xdma(b, half, ci, on_pool):
                def go():
                    st = state[b]
                    csz = min(128, C - ci * 128)
                    lo = half * 1024
                    if on_pool:
                        # GpSimd/SWDGE piece: overlap one column with the SP
                        # half-0 piece of the same tile (rewritten with the
                        # same data) so the WAW dependency queues this
                        # transfer BEHIND the critical half-0 stream instead
                        # of stealing its DMA-engine slots.
                        lo -= 1
                    dst = st["xt"][ci][:csz, lo : half * 1024 + 1024]
                    src = xt_d.ap()[
                        b, ci * 128 : ci * 128 + csz, lo : half * 1024 + 1024
                    ]
                    if on_pool:
                        nc.gpsimd.dma_start(dst, src)
                    else:
                        nc.sync.dma_start(dst, src)

                return go

            def mk_proj(b, nm, tc_, on_act):
                """qT/kT chunk: stationary W slice, moving xT; PSUM accum over C."""
                off = 0 if nm == "q" else H

                def go():
                    st = state[b]
                    dst = st[nm]
                    pp = psp.tile([128, 512], F32, tag="p", name="pp")[:96, :]
                    for ci in range(NCT):
                        csz = min(128, C - ci * 128)
                        nc.tensor.matmul(
                            pp[:, :],
                            w_sb[:csz, ci, off : off + H],
                            st["xt"][ci][:csz, tc_ * 512 : tc_ * 512 + 512],
                            start=(ci == 0),
                            stop=(ci == NCT - 1),
                        )
                    cdst = dst[:, tc_ * 512 : tc_ * 512 + 512]
                    if on_act:
                        nc.scalar.copy(cdst, pp[:, :])
                    else:
                        nc.vector.tensor_copy(cdst, pp[:, :])

                return go

            def mk_vcol(b):
                def go():
                    st = state[b]
                    vp, mf = st["vp"], st["mf"]
                    src = mf[:].rearrange("p (k o) -> p k o", o=1)
                    nc.vector.tensor_copy(vp[:, :, H : H + 1], src)
                    nc.vector.tensor_copy(vp[:, :, H + 1 : H + 2], src)

                return go

            def mk_v(b, tt):
                """v natural tile: stationary xT slice, moving Wv slice."""

                def go():
                    st = state[b]
                    pv = psp.tile([128, 512], F32, tag="p", name="pv")
                    for ci in range(NCT):
                        csz = min(128, C - ci * 128)
                        nc.tensor.matmul(
                            pv[:, :96],
                            st["xt"][ci][:csz, tt * 128 : tt * 128 + 128],
                            w_sb[:csz, ci, 2 * H : 3 * H],
                            start=(ci == 0),
                            stop=(ci == NCT - 1),
                        )
                    nc.vector.tensor_scalar_mul(
                        st["vp"][:, tt, :H], pv[:, :96], st["mf"][:, tt : tt + 1]
                    )

                return go

            def mk_score(b, qc, g):
                def go():
                    st = state[b]
                    ps = pss.tile([128, KG, 512], F32, tag="s", name="ps")
                    st["ps", qc, g] = ps
                    for j in range(KG):
                        kt = g * KG + j
                        nc.tensor.matmul(
                            ps[:, j, :],
                            st["k"][:, kt * 128 : kt * 128 + 128],
                            st["q"][:, qc * 512 : qc * 512 + 512],
                            start=True,
                            stop=True,
                        )

                return go

            def mk_exp(b, qc, g, on_pool=False):
                def go():
                    st = state[b]
                    es = es_pool.tile([128, KG, 512], BF16, tag="es", name="es")
                    st["es", qc, g] = es
                    ps = st.pop(("ps", qc, g))
                    if on_pool:
                        eng = nc.gpsimd
                        bias = nc.const_aps.scalar_like(0.0, ps[:])
                        eng.add_instruction(
                            mybir.InstActivation(
                                name=nc.get_next_instruction_name(),
                                func=exp,
                                ins=[
                                    eng.lower_ap(ps[:]),
                                    eng.lower_ap(bias),
                                    mybir.ImmediateValue(dtype=F32, value=SCALE),
                                    mybir.ImmediateValue(dtype=F32, value=0.0),
                                ],
                                outs=[eng.lower_ap(es[:])],
                            )
                        )
                    else:
                        nc.scalar.activation(es[:], ps[:], exp, scale=SCALE)

                return go

            def chain_part(st, b, qc, jq, lo, hi):
                """out-chain piece for one qt-tile: accumulating matmuls kt
                lo..hi-1 into this chain's dedicated PSUM bank (one open
                group per bank at a time); fin + (maybe) store at the end."""
                if ("po", qc, jq) not in st:
                    st["po", qc, jq] = pso.tile([128, 512], F32, tag="o", name="po")
                po = st["po", qc, jq]
                for kt in range(lo, hi):
                    es = st["es", qc, kt // KG]
                    nc.tensor.matmul(
                        po[:, :HP],
                        es[:, kt % KG, jq * 128 : jq * 128 + 128],
                        st["vp"][:, kt, :],
                        start=(kt == 0),
                        stop=(kt == NT - 1),
                    )
                if hi < NT:
                    return
                st.pop(("po", qc, jq))
                rec = st["rec", qc]
                ot = st["ot", qc]
                nc.vector.reciprocal(rec[:, jq : jq + 1], po[:, H : H + 1])
                nc.vector.tensor_scalar_mul(
                    ot[:, jq, :], po[:, :H], rec[:, jq : jq + 1]
                )
                if jq == NQC - 1:
                    for g in range(NG):
                        st.pop(("es", qc, g))
                    dst = out_d.ap()[b, qc * 512 : (qc + 1) * 512, :].rearrange(
                        "(p j) h -> p j h", j=NQC
                    )
                    nc.sync.dma_start(dst, st.pop(("ot", qc)))

            def mk_chain(b, qc, jq, lo=0, hi=NT):
                def go():
                    st = state[b]
                    if ("ot", qc) not in st:
                        st["rec", qc] = ot_pool.tile(
                            [128, NQC], F32, tag="rec", name="rec"
                        )
                        st["ot", qc] = ot_pool.tile(
                            [128, NQC, H], F32, tag="ot", name="ot"
                        )
                    chain_part(st, b, qc, jq, lo, hi)

                return go

            # ---- allocate persistent tiles ---------------------------------
            for b in range(BPC):
                state[b] = {}
                st = state[b]
                st["xt"] = [
                    xt_pool.tile([128, T], BF16, tag=f"xt{ci}", name=f"xt{ci}_{b}")
                    for ci in range(NCT)
                ]
                st["q"] = qk_pool.tile([96, T], BF16, tag="q", name=f"q{b}")
                st["k"] = qk_pool.tile([96, T], BF16, tag="k", name=f"k{b}")
                st["vp"] = vp_pool.tile([128, NT, HP], BF16, name=f"vp{b}")

            def c_phase(blocks, fills):
                """blocks: list of (b, qc). Chains of block i ride inside
                block i+1's score/exp stream (slots g=1,3,5,7); the final
                block's chains flush at the end."""
                pending = None
                for bi, (b, qc) in enumerate(blocks):
                    last = bi == len(blocks) - 1
                    mk_score(b, qc, 0)()
                    for g in range(NG):
                        mk_exp(b, qc, g, on_pool=(last and g in POOL_EXP_GROUPS))()
                        if g + 1 < NG:
                            mk_score(b, qc, g + 1)()
                        if pending is not None:
                            if not last and g % 2 == 1:
                                pending[g // 2]()
                            elif last and 1 <= g <= 4:
                                pending[g - 1]()
                        for u in fills.get((b, qc), [[]] * NG)[g]:
                            u()
                        if last and g == 5:
                            mk_chain(b, qc, 0, 0, NT // 2)()
                        if last and g == 6:
                            mk_chain(b, qc, 1, 0, NT // 2)()
                    if last:
                        mk_chain(b, qc, 0, NT // 2, NT)()
                        mk_chain(b, qc, 1, NT // 2, NT)()
                        mk_chain(b, qc, 2)()
                        mk_chain(b, qc, 3)()
                    else:
                        pending = [mk_chain(b, qc, jq) for jq in range(NQC)]

            def run(units):
                for u in units:
                    u()

            def P(b, nm, tc_, on_act=False):
                return mk_proj(b, nm, tc_, on_act)

            def V(b, *tts):
                return [mk_v(b, tt) for tt in tts]

            # ---- emission ---------------------------------------------------
            # DMA order: W first (needed by every matmul), then b0 x half 0
            # (kT0/qT0 critical path), mask 0, b0 x half 1, mask 1.
            # b0 x on SP/HWDGE, b1 x on GpSimd/SWDGE (parallel desc-gen).
            for ci in range(NCT):
                mk_xdma(0, 0, ci, on_pool=False)()
            mk_mask(0)()
            for ci in range(NCT):
                mk_xdma(0, 1, ci, on_pool=False)()
            mk_mask(1)()
            for half in range(2):
                for ci in range(NCT):
                    mk_xdma(1, half, ci, on_pool=False)()

            # b0 head: kT0 and qT0 matmuls interleaved per c-tile so both
            # PSUM groups fill as x arrives; copies land on ACT (k) and DVE
            # (q) in parallel. Then mask col + first v tiles for out(qc0,0).
            ppk = psp.tile([128, 512], F32, tag="p", name="ppk")[:96, :]
            ppq = psp.tile([128, 512], F32, tag="p", name="ppq")[:96, :]
            pkh = pso.tile([128, 512], F32, tag="o", name="pkh")[:96, :]
            st0 = state[0]
            # kT cols 0-255 in a separate half-group so score(qc0, g0) can
            # start before the rest of chunk 0 lands; q0 full (scores move
            # the whole 512-wide qT chunk).
            for ci in range(NCT):
                csz = min(128, C - ci * 128)
                for pp, off, w0 in ((pkh, H, 0), (ppk, H, 256), (ppq, 0, 0)):
                    wid = 256 if pp is not ppq else 512
                    nc.tensor.matmul(
                        pp[:, :wid],
                        w_sb[:csz, ci, off : off + H],
                        st0["xt"][ci][:csz, w0 : w0 + wid],
                        start=(ci == 0),
                        stop=(ci == NCT - 1),
                    )
            nc.scalar.copy(st0["k"][:, 0:256], pkh[:, :256])
            nc.vector.tensor_copy(st0["q"][:, 0:512], ppq[:, :])
            nc.scalar.copy(st0["k"][:, 256:512], ppk[:, :256])
            run([mk_vcol(0)] + V(0, 0, 1))

            # Fill ledger: score(qc,g) needs kT chunk g//2 by slot g-2 and
            # qT chunk qc; out(qc,g) needs v tiles {2g,2g+1} by slot g.
            fills0 = [
                [
                    [P(0, "k", 1)],
                    V(0, 2, 3),
                    [P(0, "k", 2)] + V(0, 4, 5),
                    V(0, 6, 7),
                    [P(0, "k", 3)] + V(0, 8, 9),
                    V(0, 10, 11),
                    [P(0, "q", 1)] + V(0, 12, 13),
                    V(0, 14, 15),
                ],
                [
                    [P(0, "q", 2)],
                    [],
                    [P(1, "k", 0)],
                    [],
                    [P(1, "q", 0)],
                    [mk_vcol(1)],
                    [P(0, "q", 3)],
                    V(1, 0, 1),
                ],
                [
                    [P(1, "k", 1)],
                    V(1, 2, 3),
                    [],
                    V(1, 4, 5),
                    [],
                    [],
                    [],
                    [],
                ],
                [
                    [P(1, "k", 2)],
                    V(1, 6, 7),
                    [],
                    V(1, 8, 9),
                    [],
                    [],
                    [],
                    [],
                ],
            ]
            # b1's remaining prep rides just-in-time in its own fill
            # slots (k'3 before score g6, v' pairs before their out group,
            # q'1..3 before their qc).
            E = []
            fills1 = [
                [
                    [P(1, "k", 3)],
                    V(1, 10, 11),
                    V(1, 12, 13),
                    V(1, 14, 15),
                    [],
                    [],
                    [P(1, "q", 1)],
                    [],
                ],
                [[P(1, "q", 2)], E, E, E, E, E, E, E],
                [[P(1, "q", 3)], E, E, E, E, E, E, E],
                [E] * NG,
            ]
            fills = {}
            for qc in range(NQC):
                fills[(0, qc)] = fills0[qc]
                fills[(1, qc)] = fills1[qc]
            blocks = [(b, qc) for b in range(BPC) for qc in range(NQC)]
            c_phase(blocks, fills)

    _split_excess_waits(nc)
    return nc


_prog = None


def _get_prog():
    global _prog
    if _prog is None:
        _prog = _build()
    return _prog


def kernel(x, mask, Wk, Wq, Wv, **_ignored):
    bf16 = ml_dtypes.bfloat16
    # host-side prep: transpose x, cast to bf16, pack weights (zero-pad C)
    xt = np.ascontiguousarray(
        np.asarray(x, dtype=np.float32).transpose(0, 2, 1)
    ).astype(bf16)
    w = np.zeros((NCT * 128, 3 * H), dtype=bf16)
    w[:C] = np.concatenate(
        [np.asarray(Wq), np.asarray(Wk), np.asarray(Wv)], axis=1
    ).astype(bf16)
    maskf = np.asarray(mask).astype(np.float32).reshape(B, NT, 128).transpose(0, 2, 1)
    maskf = np.ascontiguousarray(maskf)

    nc = _get_prog()
    in_maps = [
        {
            "xt": xt[i * BPC : (i + 1) * BPC],
            "w": w,
            "maskf": maskf[i * BPC : (i + 1) * BPC],
        }
        for i in range(N_CORES)
    ]
    res = run_bass_kernel_spmd(nc, in_maps, core_ids=list(range(N_CORES)))
    raw = np.concatenate([res.results[i]["out"] for i in range(N_CORES)], axis=0)
    # un-permute the store layout: dram row qc*512 + 4p+j holds q = qc*512+128j+p
    out = raw.reshape(B, NQC, 128, 4, H).transpose(0, 1, 3, 2, 4).reshape(B, T, H)
    return np.ascontiguousarray(out)


if __name__ == "__main__":
    rng = np.random.default_rng(0)
    x = rng.standard_normal((B, T, C), dtype=np.float32)
    mask = np.ones((B, T), dtype=bool)
    s = 1.0 / np.sqrt(C)
    Wk = (rng.standard_normal((C, H)) * s).astype(np.float32)
    Wq = (rng.standard_normal((C, H)) * s).astype(np.float32)
    Wv = (rng.standard_normal((C, H)) * s).astype(np.float32)
    out = kernel(x, mask=mask, Wk=Wk, Wq=Wq, Wv=Wv)
    print("out", out.shape, out.dtype, float(np.abs(out).max()))


# revision 43
# speedup vs baseline: 1.0016x; 1.0016x over previous
"""Single-head attention (B=16, T=2048, C=576, H=96) on 8 TRN2 NeuronCores.

Sharding: data-parallel over batch — 2 batches per core; weights replicated.

All matmul operands are bf16 (fp32 PSUM accumulation); rel-err budget is
2e-2 and bf16 end-to-end measures ~7e-3. The host pre-transposes x to
xT=[C,T] bf16 and packs W=[5,128,288] (zero-padded C) bf16, so the device
never transposes anything:

  qT,kT [96,T]  = W-slice stationary, xT moving          (PSUM-accum over C)
  v_nat [T,96]  = xT-tile stationary, Wv-slice moving    (natural layout)
  vp [128,NT,98]: v*mask with mask in cols 96,97 (denominator trick)
  sT [kpos,qpos] per (qc, 2-kt group) into a 2-bank PSUM tile (bufs=2) so
     the next group's score matmuls overlap the previous group's exp
  out_nat [qpos,98] = es-slice stationary, vp moving, accumulated over kt;
     col 96 is the softmax denominator. DVE reciprocal+scale, DMA out with
     a (p j) permutation for 1536B descriptors; host un-permutes.

All x loads ride the SP/HWDGE queue in priority order (b0 half 0 first);
the tiny mask loads go through the idle GpSimd engine's SWDGE so they do
not consume SP issue slots.

This walrus build rejects >1 sync wait per instruction (and any wait on a
Drain), so after TileContext builds the module we hoist excess waits onto
injected same-engine NOPs — semantics identical since engines execute
their stream in order.
"""

import sys

if "/opt/trn_rl_repo" not in sys.path:
    sys.path.insert(0, "/opt/trn_rl_repo")

import ml_dtypes
import numpy as np

import concourse.bass as bass
import concourse.tile as tile
from concourse import mybir
from concourse.bass_utils import run_bass_kernel_spmd

N_CORES = 8
B, T, C, H = 16, 2048, 576, 96
BPC = B // N_CORES  # batches per core
SCALE = 1.0 / float(np.sqrt(H))

F32 = mybir.dt.float32
BF16 = mybir.dt.bfloat16

NT = T // 128  # 16 key tiles
NCT = (C + 127) // 128  # 5 c-tiles (last is 64)
NQC = T // 512  # 4 query chunks
NG = 8  # kt-groups per query chunk (2 kt each)
KG = NT // NG  # 2 kt per group
HP = H + 2  # 98: H + denominator col + dup (even moving count)
POOL_EXP_GROUPS = ()  # which kt-groups' exp runs on GpSimd instead of ACT


def _split_excess_waits(nc, max_waits=1):
    """Hoist sync waits beyond this walrus's per-instruction limit onto
    injected NOPs that run just before, on the same engine."""
    n_split = 0
    for fn in nc.m.functions:
        for blk in fn.blocks:
            new_insts = []
            changed = False
            for inst in blk.instructions:
                si = inst.sync_info
                waits = list(si.on_wait) if si is not None else []
                cap = 0 if isinstance(inst, mybir.InstDrain) else max_waits
                if len(waits) > cap:
                    # Keep the most meaningful wait ON the instruction (its
                    # engine-stage wait doesn't block the sequencer); push
                    # self-engine sems (trivially satisfied in-order) and
                    # DMA-completion WARs onto the NOPs, which DO block SEQ.
                    eng = str(inst.engine).split(".")[-1].split(":")[0].strip("'\" >")

                    def prio(iw):
                        i, w = iw
                        nm = getattr(w, "ant_name", "") or ""
                        self_sem = nm.startswith(eng)
                        dma_sem = nm.startswith("DMAHW") or nm.startswith("DMASW")
                        return (0 if self_sem else (1 if dma_sem else 2), i)

                    order = sorted(enumerate(waits), key=prio)
                    waits = [w for _, w in order]
                    excess = waits[:-cap] if cap else waits
                    keep = waits[-cap:] if cap else []
                    for i in range(0, len(excess), max_waits):
                        chunk = excess[i : i + max_waits]
                        new_insts.append(
                            mybir.InstNoOp(
                                name=f"{inst.name}-wsplit{i}",
                                engine=inst.engine,
                                ins=[],
                                outs=[],
                                sync_info=mybir.SyncInfo(on_wait=chunk, on_update=[]),
                            )
                        )
                    inst.sync_info = mybir.SyncInfo(
                        on_wait=keep, on_update=list(si.on_update)
                    )
                    changed = True
                    n_split += 1
                new_insts.append(inst)
            if changed:
                blk.instructions = new_insts
    return n_split


def _build():
    nc = bass.Bass("TRN2", target_bir_lowering=False, debug=False)

    xt_d = nc.dram_tensor("xt", [BPC, C, T], BF16, kind="ExternalInput")
    w_d = nc.dram_tensor("w", [NCT * 128, 3 * H], BF16, kind="ExternalInput")
    mf_d = nc.dram_tensor("maskf", [BPC, 128, NT], F32, kind="ExternalInput")
    out_d = nc.dram_tensor("out", [BPC, T, H], F32, kind="ExternalOutput")

    exp = mybir.ActivationFunctionType.Exp

    with tile.TileContext(nc) as tc:
        with (
            tc.tile_pool(name="const", bufs=1) as const_pool,
            tc.tile_pool(name="xt", bufs=2) as xt_pool,
            tc.tile_pool(name="qk", bufs=2) as qk_pool,
            tc.tile_pool(name="vp", bufs=2) as vp_pool,
            tc.tile_pool(name="mk", bufs=2) as mk_pool,
            tc.tile_pool(name="es", bufs=17) as es_pool,
            tc.tile_pool(name="ot", bufs=4) as ot_pool,
            tc.tile_pool(name="psx", bufs=2, space="PSUM") as psx,  # proj+v
            tc.tile_pool(name="pso", bufs=2, space="PSUM") as pso,  # out chains
            tc.tile_pool(name="pss", bufs=2, space="PSUM") as pss,  # scores 2-bank
        ):
            # PE p-state warm-up: dependency-free dummy matmuls on garbage
            # SBUF data while the head DMAs land (PE is idle anyway). After
            # ~3us of continuous busy the cost model (and hardware) runs the
            # array at full clock, so the first real matmuls aren't 2x slow.
            # The psum bank is reset by its next user's start=True matmul.
            dmy = const_pool.tile([128, 640], BF16, name="dmy")
            nc.vector.memset(dmy[:], 0)
            dpo = pso.tile([128, 512], F32, tag="o", name="dpo")
            for _ in range(1):
                nc.tensor.matmul(
                    dpo[:, :],
                    dmy[:, 0:128],
                    dmy[:, 128:640],
                    start=True,
                    stop=True,
                )

            w_sb = const_pool.tile([128, NCT, 3 * H], BF16, name="w_sb")
            nc.sync.dma_start(
                w_sb[:], w_d.ap().rearrange("(g p) c -> p g c", p=128)
            )
            # pre-warm the exp table so the first real exp doesn't pay the
            # table load inside the pipeline
            warm = const_pool.tile([128, 2], F32, name="warm")
            nc.scalar.activation(warm[:], w_sb[:, 0, 0:4].bitcast(F32), exp)

            state = {}

            def mk_mask(b):
                def go():
                    mf = mk_pool.tile([128, NT], F32, name=f"mf{b}")
                    state[b]["mf"] = mf
                    nc.gpsimd.dma_start(mf[:], mf_d.ap()[b])

                return go

            def mk_xdma(b, half, ci, on_pool):
                def go():
                    st = state[b]
                    csz = min(128, C - ci * 128)
                    lo = half * 1024
                    if on_pool:
                        # GpSimd/SWDGE piece: overlap one column with the SP
                        # half-0 piece of the same tile (rewritten with the
                        # same data) so the WAW dependency queues this
                        # transfer BEHIND the critical half-0 stream instead
                        # of stealing its DMA-engine slots.
                        lo -= 1
                    dst = st["xt"][ci][:csz, lo : half * 1024 + 1024]
                    src = xt_d.ap()[
                        b, ci * 128 : ci * 128 + csz, lo : half * 1024 + 1024
                    ]
                    if on_pool:
                        nc.gpsimd.dma_start(dst, src)
                    else:
                        nc.sync.dma_start(dst, src)

                return go

            def mk_proj(b, nm, tc_, on_act):
                """qT/kT chunk: stationary W slice, moving xT; PSUM accum over C."""
                off = 0 if nm == "q" else H

                def go():
                    st = state[b]
                    dst = st[nm]
                    pp = psx.tile([128, 512], F32, tag="p", name="pp")[:96, :]
                    for ci in range(NCT):
                        csz = min(128, C - ci * 128)
                        nc.tensor.matmul(
                            pp[:, :],
                            w_sb[:csz, ci, off : off + H],
                            st["xt"][ci][:csz, tc_ * 512 : tc_ * 512 + 512],
                            start=(ci == 0),
                            stop=(ci == NCT - 1),
                        )
                    cdst = dst[:, tc_ * 512 : tc_ * 512 + 512]
                    if on_act:
                        nc.scalar.copy(cdst, pp[:, :])
                    else:
                        nc.vector.tensor_copy(cdst, pp[:, :])

                return go

            def mk_vcol(b):
                def go():
                    st = state[b]
                    vp, mf = st["vp"], st["mf"]
                    src = mf[:].rearrange("p (k o) -> p k o", o=1)
                    nc.vector.tensor_copy(vp[:, :, H : H + 1], src)
                    nc.vector.tensor_copy(vp[:, :, H + 1 : H + 2], src)

                return go

            def mk_v(b, tt):
                """v natural tile: stationary xT slice, moving Wv slice."""

                def go():
                    st = state[b]
                    pv = psx.tile([128, 512], F32, tag="p", name="pv")
                    for ci in range(NCT):
                        csz = min(128, C - ci * 128)
                        nc.tensor.matmul(
                            pv[:, :96],
                            st["xt"][ci][:csz, tt * 128 : tt * 128 + 128],
                            w_sb[:csz, ci, 2 * H : 3 * H],
                            start=(ci == 0),
                            stop=(ci == NCT - 1),
                        )
                    nc.vector.tensor_scalar_mul(
                        st["vp"][:, tt, :H], pv[:, :96], st["mf"][:, tt : tt + 1]
                    )

                return go

            def mk_score(b, qc, g):
                def go():
                    st = state[b]
                    ps = pss.tile([128, KG, 512], F32, tag="s", name="ps")
                    st["ps", qc, g] = ps
                    for j in range(KG):
                        kt = g * KG + j
                        nc.tensor.matmul(
                            ps[:, j, :],
                            st["k"][:, kt * 128 : kt * 128 + 128],
                            st["q"][:, qc * 512 : qc * 512 + 512],
                            start=True,
                            stop=True,
                        )

                return go

            def mk_exp(b, qc, g, on_pool=False):
                def go():
                    st = state[b]
                    es = es_pool.tile([128, KG, 512], BF16, tag="es", name="es")
                    st["es", qc, g] = es
                    ps = st.pop(("ps", qc, g))
                    if on_pool:
                        eng = nc.gpsimd
                        bias = nc.const_aps.scalar_like(0.0, ps[:])
                        eng.add_instruction(
                            mybir.InstActivation(
                                name=nc.get_next_instruction_name(),
                                func=exp,
                                ins=[
                                    eng.lower_ap(ps[:]),
                                    eng.lower_ap(bias),
                                    mybir.ImmediateValue(dtype=F32, value=SCALE),
                                    mybir.ImmediateValue(dtype=F32, value=0.0),
                                ],
                                outs=[eng.lower_ap(es[:])],
                            )
                        )
                    else:
                        nc.scalar.activation(es[:], ps[:], exp, scale=SCALE)

                return go

            def chain_part(st, b, qc, jq, lo, hi):
                """out-chain piece for one qt-tile: accumulating matmuls kt
                lo..hi-1 into this chain's dedicated PSUM bank (one open
                group per bank at a time); fin + (maybe) store at the end."""
                if ("po", qc, jq) not in st:
                    st["po", qc, jq] = pso.tile([128, 512], F32, tag="o", name="po")
                po = st["po", qc, jq]
                for kt in range(lo, hi):
                    es = st["es", qc, kt // KG]
                    nc.tensor.matmul(
                        po[:, :HP],
                        es[:, kt % KG, jq * 128 : jq * 128 + 128],
                        st["vp"][:, kt, :],
                        start=(kt == 0),
                        stop=(kt == NT - 1),
                    )
                if hi < NT:
                    return
                st.pop(("po", qc, jq))
                rec = st["rec", qc]
                ot = st["ot", qc]
                nc.vector.reciprocal(rec[:, jq : jq + 1], po[:, H : H + 1])
                nc.vector.tensor_scalar_mul(
                    ot[:, jq, :], po[:, :H], rec[:, jq : jq + 1]
                )
                if jq == NQC - 1:
                    for g in range(NG):
                        st.pop(("es", qc, g))
                    dst = out_d.ap()[b, qc * 512 : (qc + 1) * 512, :].rearrange(
                        "(p j) h -> p j h", j=NQC
                    )
                    nc.sync.dma_start(dst, st.pop(("ot", qc)))

            def mk_chain(b, qc, jq, lo=0, hi=NT):
                def go():
                    st = state[b]
                    if ("ot", qc) not in st:
                        st["rec", qc] = ot_pool.tile(
                            [128, NQC], F32, tag="rec", name="rec"
                        )
                        st["ot", qc] = ot_pool.tile(
                            [128, NQC, H], F32, tag="ot", name="ot"
                        )
                    chain_part(st, b, qc, jq, lo, hi)

                return go

            # ---- allocate persistent tiles ---------------------------------
            for b in range(BPC):
                state[b] = {}
                st = state[b]
                st["xt"] = [
                    xt_pool.tile([128, T], BF16, tag=f"xt{ci}", name=f"xt{ci}_{b}")
                    for ci in range(NCT)
                ]
                st["q"] = qk_pool.tile([96, T], BF16, tag="q", name=f"q{b}")
                st["k"] = qk_pool.tile([96, T], BF16, tag="k", name=f"k{b}")
                st["vp"] = vp_pool.tile([128, NT, HP], BF16, name=f"vp{b}")

            def c_phase(blocks, fills):
                """blocks: list of (b, qc). Chains of block i ride inside
                block i+1's score/exp stream (slots g=1,3,5,7); the final
                block's chains flush at the end."""
                pending = None
                for bi, (b, qc) in enumerate(blocks):
                    last = bi == len(blocks) - 1
                    mk_score(b, qc, 0)()
                    for g in range(NG):
                        mk_exp(b, qc, g, on_pool=(last and g in POOL_EXP_GROUPS))()
                        if g + 1 < NG:
                            mk_score(b, qc, g + 1)()
                        if pending is not None:
                            if not last and g % 2 == 1:
                                pending[g // 2]()
                            elif last and 1 <= g <= 4:
                                pending[g - 1]()
                        for u in fills.get((b, qc), [[]] * NG)[g]:
                            u()
                        if last and g == 5:
                            mk_chain(b, qc, 0, 0, NT // 2)()
                        if last and g == 6:
                            mk_chain(b, qc, 1, 0, NT // 2)()
                    if last:
                        mk_chain(b, qc, 0, NT // 2, NT)()
                        mk_chain(b, qc, 1, NT // 2, NT)()
                        mk_chain(b, qc, 2)()
                        mk_chain(b, qc, 3)()
                    else:
                        pending = [mk_chain(b, qc, jq) for jq in range(NQC)]

            def run(units):
                for u in units:
                    u()

            def P(b, nm, tc_, on_act=False):
                return mk_proj(b, nm, tc_, on_act)

            def V(b, *tts):
                return [mk_v(b, tt) for tt in tts]

            # ---- emission ---------------------------------------------------
            # DMA order: W first (needed by every matmul), then b0 x half 0
            # (kT0/qT0 critical path), mask 0, b0 x half 1, mask 1.
            # b0 x on SP/HWDGE, b1 x on GpSimd/SWDGE (parallel desc-gen).
            for ci in range(NCT):
                mk_xdma(0, 0, ci, on_pool=False)()
            mk_mask(0)()
            for ci in range(NCT):
                mk_xdma(0, 1, ci, on_pool=False)()
            mk_mask(1)()
            for half in range(2):
                for ci in range(NCT):
                    mk_xdma(1, half, ci, on_pool=False)()

            # b0 head: kT0 and qT0 matmuls interleaved per c-tile so both
            # PSUM groups fill as x arrives; copies land on ACT (k) and DVE
            # (q) in parallel. Then mask col + first v tiles for out(qc0,0).
            ppk = psx.tile([128, 512], F32, tag="p", name="ppk")[:96, :]
            ppq = psx.tile([128, 512], F32, tag="p", name="ppq")[:96, :]
            st0 = state[0]
            for ci in range(NCT):
                csz = min(128, C - ci * 128)
                for pp, off in ((ppk, H), (ppq, 0)):
                    nc.tensor.matmul(
                        pp[:, :],
                        w_sb[:csz, ci, off : off + H],
                        st0["xt"][ci][:csz, 0:512],
                        start=(ci == 0),
                        stop=(ci == NCT - 1),
                    )
            nc.scalar.copy(st0["k"][:, 0:512], ppk[:, :])
            nc.vector.tensor_copy(st0["q"][:, 0:512], ppq[:, :])
            run([mk_vcol(0)] + V(0, 0, 1))

            # Fill ledger: score(qc,g) needs kT chunk g//2 by slot g-2 and
            # qT chunk qc; out(qc,g) needs v tiles {2g,2g+1} by slot g.
            fills0 = [
                [
                    [P(0, "k", 1)],
                    V(0, 2, 3),
                    [P(0, "k", 2)] + V(0, 4, 5),
                    V(0, 6, 7),
                    [P(0, "k", 3)] + V(0, 8, 9),
                    V(0, 10, 11),
                    [P(0, "q", 1)] + V(0, 12, 13),
                    V(0, 14, 15),
                ],
                [
                    [P(0, "q", 2)],
                    [],
                    [P(1, "k", 0)],
                    [],
                    [P(1, "q", 0)],
                    [mk_vcol(1)],
                    [P(0, "q", 3)],
                    V(1, 0, 1),
                ],
                [
                    [P(1, "k", 1)],
                    V(1, 2, 3),
                    [],
                    V(1, 4, 5),
                    [],
                    [],
                    [],
                    [],
                ],
                [
                    [P(1, "k", 2)],
                    V(1, 6, 7),
                    [],
                    V(1, 8, 9),
                    [],
                    [],
                    [],
                    [],
                ],
            ]
            # b1's remaining prep rides just-in-time in its own fill
            # slots (k'3 before score g6, v' pairs before their out group,
            # q'1..3 before their qc).
            E = []
            fills1 = [
                [
                    [P(1, "k", 3)],
                    V(1, 10, 11),
                    V(1, 12, 13),
                    V(1, 14, 15),
                    [],
                    [],
                    [P(1, "q", 1)],
                    [],
                ],
                [[P(1, "q", 2)], E, E, E, E, E, E, E],
                [[P(1, "q", 3)], E, E, E, E, E, E, E],
                [E] * NG,
            ]
            fills = {}
            for qc in range(NQC):
                fills[(0, qc)] = fills0[qc]
                fills[(1, qc)] = fills1[qc]
            blocks = [(b, qc) for b in range(BPC) for qc in range(NQC)]
            c_phase(blocks, fills)

    _split_excess_waits(nc)
    return nc


_prog = None


def _get_prog():
    global _prog
    if _prog is None:
        _prog = _build()
    return _prog


def kernel(x, mask, Wk, Wq, Wv, **_ignored):
    bf16 = ml_dtypes.bfloat16
    # host-side prep: transpose x, cast to bf16, pack weights (zero-pad C)
    xt = np.ascontiguousarray(
        np.asarray(x, dtype=np.float32).transpose(0, 2, 1)
    ).astype(bf16)
    w = np.zeros((NCT * 128, 3 * H), dtype=bf16)
    w[:C] = np.concatenate(
        [np.asarray(Wq), np.asarray(Wk), np.asarray(Wv)], axis=1
    ).astype(bf16)
    maskf = np.asarray(mask).astype(np.float32).reshape(B, NT, 128).transpose(0, 2, 1)
    maskf = np.ascontiguousarray(maskf)

    nc = _get_prog()
    in_maps = [
        {
            "xt": xt[i * BPC : (i + 1) * BPC],
            "w": w,
            "maskf": maskf[i * BPC : (i + 1) * BPC],
        }
        for i in range(N_CORES)
    ]
    res = run_bass_kernel_spmd(nc, in_maps, core_ids=list(range(N_CORES)))
    raw = np.concatenate([res.results[i]["out"] for i in range(N_CORES)], axis=0)
    # un-permute the store layout: dram row qc*512 + 4p+j holds q = qc*512+128j+p
    out = raw.reshape(B, NQC, 128, 4, H).transpose(0, 1, 3, 2, 4).reshape(B, T, H)
    return np.ascontiguousarray(out)


if __name__ == "__main__":
    rng = np.random.default_rng(0)
    x = rng.standard_normal((B, T, C), dtype=np.float32)
    mask = np.ones((B, T), dtype=bool)
    s = 1.0 / np.sqrt(C)
    Wk = (rng.standard_normal((C, H)) * s).astype(np.float32)
    Wq = (rng.standard_normal((C, H)) * s).astype(np.float32)
    Wv = (rng.standard_normal((C, H)) * s).astype(np.float32)
    out = kernel(x, mask=mask, Wk=Wk, Wq=Wq, Wv=Wv)
    print("out", out.shape, out.dtype, float(np.abs(out).max()))


# revision 46
# speedup vs baseline: 1.0106x; 1.0090x over previous
"""Single-head attention (B=16, T=2048, C=576, H=96) on 8 TRN2 NeuronCores.

Sharding: data-parallel over batch — 2 batches per core; weights replicated.

All matmul operands are bf16 (fp32 PSUM accumulation); rel-err budget is
2e-2 and bf16 end-to-end measures ~7e-3. The host pre-transposes x to
xT=[C,T] bf16 and packs W=[5,128,288] (zero-padded C) bf16, so the device
never transposes anything:

  qT,kT [96,T]  = W-slice stationary, xT moving          (PSUM-accum over C)
  v_nat [T,96]  = xT-tile stationary, Wv-slice moving    (natural layout)
  vp [128,NT,98]: v*mask with mask in cols 96,97 (denominator trick)
  sT [kpos,qpos] per (qc, 2-kt group) into a 2-bank PSUM tile (bufs=2) so
     the next group's score matmuls overlap the previous group's exp
  out_nat [qpos,98] = es-slice stationary, vp moving, accumulated over kt;
     col 96 is the softmax denominator. DVE reciprocal+scale, DMA out with
     a (p j) permutation for 1536B descriptors; host un-permutes.

All x loads ride the SP/HWDGE queue in priority order (b0 half 0 first);
the tiny mask loads go through the idle GpSimd engine's SWDGE so they do
not consume SP issue slots.

This walrus build rejects >1 sync wait per instruction (and any wait on a
Drain), so after TileContext builds the module we hoist excess waits onto
injected same-engine NOPs — semantics identical since engines execute
their stream in order.
"""

import sys

if "/opt/trn_rl_repo" not in sys.path:
    sys.path.insert(0, "/opt/trn_rl_repo")

import ml_dtypes
import numpy as np

import concourse.bass as bass
import concourse.tile as tile
from concourse import mybir
from concourse.bass_utils import run_bass_kernel_spmd

N_CORES = 8
B, T, C, H = 16, 2048, 576, 96
BPC = B // N_CORES  # batches per core
SCALE = 1.0 / float(np.sqrt(H))

F32 = mybir.dt.float32
BF16 = mybir.dt.bfloat16

NT = T // 128  # 16 key tiles
NCT = (C + 127) // 128  # 5 c-tiles (last is 64)
NQC = T // 512  # 4 query chunks
NG = 8  # kt-groups per query chunk (2 kt each)
KG = NT // NG  # 2 kt per group
HP = H + 2  # 98: H + denominator col + dup (even moving count)
POOL_EXP_GROUPS = ()  # which kt-groups' exp runs on GpSimd instead of ACT


def _split_excess_waits(nc, max_waits=1):
    """Hoist sync waits beyond this walrus's per-instruction limit onto
    injected NOPs that run just before, on the same engine."""
    n_split = 0
    for fn in nc.m.functions:
        for blk in fn.blocks:
            new_insts = []
            changed = False
            for inst in blk.instructions:
                si = inst.sync_info
                waits = list(si.on_wait) if si is not None else []
                cap = 0 if isinstance(inst, mybir.InstDrain) else max_waits
                if len(waits) > cap:
                    # Keep the most meaningful wait ON the instruction (its
                    # engine-stage wait doesn't block the sequencer); push
                    # self-engine sems (trivially satisfied in-order) and
                    # DMA-completion WARs onto the NOPs, which DO block SEQ.
                    eng = str(inst.engine).split(".")[-1].split(":")[0].strip("'\" >")

                    def prio(iw):
                        i, w = iw
                        nm = getattr(w, "ant_name", "") or ""
                        self_sem = nm.startswith(eng)
                        dma_sem = nm.startswith("DMAHW") or nm.startswith("DMASW")
                        return (0 if self_sem else (1 if dma_sem else 2), i)

                    order = sorted(enumerate(waits), key=prio)
                    waits = [w for _, w in order]
                    excess = waits[:-cap] if cap else waits
                    keep = waits[-cap:] if cap else []
                    for i in range(0, len(excess), max_waits):
                        chunk = excess[i : i + max_waits]
                        new_insts.append(
                            mybir.InstNoOp(
                                name=f"{inst.name}-wsplit{i}",
                                engine=inst.engine,
                                ins=[],
                                outs=[],
                                sync_info=mybir.SyncInfo(on_wait=chunk, on_update=[]),
                            )
                        )
                    inst.sync_info = mybir.SyncInfo(
                        on_wait=keep, on_update=list(si.on_update)
                    )
                    changed = True
                    n_split += 1
                new_insts.append(inst)
            if changed:
                blk.instructions = new_insts
    return n_split


def _build():
    nc = bass.Bass("TRN2", target_bir_lowering=False, debug=False)

    xt_d = nc.dram_tensor("xt", [BPC, C, T], BF16, kind="ExternalInput")
    w_d = nc.dram_tensor("w", [NCT * 128, 3 * H], BF16, kind="ExternalInput")
    mf_d = nc.dram_tensor("maskf", [BPC, 128, NT], F32, kind="ExternalInput")
    out_d = nc.dram_tensor("out", [BPC, T, H], F32, kind="ExternalOutput")

    exp = mybir.ActivationFunctionType.Exp

    with tile.TileContext(nc) as tc:
        with (
            tc.tile_pool(name="const", bufs=1) as const_pool,
            tc.tile_pool(name="xt", bufs=2) as xt_pool,
            tc.tile_pool(name="qk", bufs=2) as qk_pool,
            tc.tile_pool(name="vp", bufs=2) as vp_pool,
            tc.tile_pool(name="mk", bufs=2) as mk_pool,
            tc.tile_pool(name="es", bufs=17) as es_pool,
            tc.tile_pool(name="ot", bufs=4) as ot_pool,
            tc.tile_pool(name="psx", bufs=2, space="PSUM") as psx,  # proj+v
            tc.tile_pool(name="pso", bufs=2, space="PSUM") as pso,  # out chains
            tc.tile_pool(name="pss", bufs=2, space="PSUM") as pss,  # scores 2-bank
        ):
            # PE p-state warm-up: dependency-free dummy matmuls on garbage
            # SBUF data while the head DMAs land (PE is idle anyway). After
            # ~3us of continuous busy the cost model (and hardware) runs the
            # array at full clock, so the first real matmuls aren't 2x slow.
            # The psum bank is reset by its next user's start=True matmul.
            dmy = const_pool.tile([128, 640], BF16, name="dmy")
            nc.vector.memset(dmy[:], 0)
            dpo = pso.tile([128, 512], F32, tag="o", name="dpo")
            for _ in range(1):
                nc.tensor.matmul(
                    dpo[:, :],
                    dmy[:, 0:128],
                    dmy[:, 128:640],
                    start=True,
                    stop=True,
                )

            w_sb = const_pool.tile([128, NCT, 3 * H], BF16, name="w_sb")
            nc.sync.dma_start(
                w_sb[:], w_d.ap().rearrange("(g p) c -> p g c", p=128)
            )
            # pre-warm the exp table so the first real exp doesn't pay the
            # table load inside the pipeline
            warm = const_pool.tile([128, 2], F32, name="warm")
            nc.scalar.activation(warm[:], w_sb[:, 0, 0:4].bitcast(F32), exp)

            state = {}

            def mk_mask(b):
                def go():
                    mf = mk_pool.tile([128, NT], F32, name=f"mf{b}")
                    state[b]["mf"] = mf
                    nc.gpsimd.dma_start(mf[:], mf_d.ap()[b])

                return go

            def mk_xdma(b, half, ci, on_pool):
                def go():
                    st = state[b]
                    csz = min(128, C - ci * 128)
                    lo = half * 1024
                    if on_pool:
                        # GpSimd/SWDGE piece: overlap one column with the SP
                        # half-0 piece of the same tile (rewritten with the
                        # same data) so the WAW dependency queues this
                        # transfer BEHIND the critical half-0 stream instead
                        # of stealing its DMA-engine slots.
                        lo -= 1
                    dst = st["xt"][ci][:csz, lo : half * 1024 + 1024]
                    src = xt_d.ap()[
                        b, ci * 128 : ci * 128 + csz, lo : half * 1024 + 1024
                    ]
                    if on_pool:
                        nc.gpsimd.dma_start(dst, src)
                    else:
                        nc.sync.dma_start(dst, src)

                return go

            def mk_proj(b, nm, tc_, on_act):
                """qT/kT chunk: stationary W slice, moving xT; PSUM accum over C."""
                off = 0 if nm == "q" else H

                def go():
                    st = state[b]
                    dst = st[nm]
                    pp = psx.tile([128, 512], F32, tag="p", name="pp")[:96, :]
                    for ci in range(NCT):
                        csz = min(128, C - ci * 128)
                        nc.tensor.matmul(
                            pp[:, :],
                            w_sb[:csz, ci, off : off + H],
                            st["xt"][ci][:csz, tc_ * 512 : tc_ * 512 + 512],
                            start=(ci == 0),
                            stop=(ci == NCT - 1),
                        )
                    if nm == "k":
                        # split the copy so the first half unblocks its
                        # score group before the second half drains
                        for h2 in range(2):
                            cdst = dst[:, tc_ * 512 + h2 * 256 : tc_ * 512 + h2 * 256 + 256]
                            csrc = pp[:, h2 * 256 : h2 * 256 + 256]
                            if on_act:
                                nc.scalar.copy(cdst, csrc)
                            else:
                                nc.vector.tensor_copy(cdst, csrc)
                    else:
                        cdst = dst[:, tc_ * 512 : tc_ * 512 + 512]
                        if on_act:
                            nc.scalar.copy(cdst, pp[:, :])
                        else:
                            nc.vector.tensor_copy(cdst, pp[:, :])

                return go

            def mk_vcol(b):
                def go():
                    st = state[b]
                    vp, mf = st["vp"], st["mf"]
                    src = mf[:].rearrange("p (k o) -> p k o", o=1)
                    nc.vector.tensor_copy(vp[:, :, H : H + 1], src)
                    nc.vector.tensor_copy(vp[:, :, H + 1 : H + 2], src)

                return go

            def mk_v(b, tt):
                """v natural tile: stationary xT slice, moving Wv slice."""

                def go():
                    st = state[b]
                    pv = psx.tile([128, 512], F32, tag="p", name="pv")
                    for ci in range(NCT):
                        csz = min(128, C - ci * 128)
                        nc.tensor.matmul(
                            pv[:, :96],
                            st["xt"][ci][:csz, tt * 128 : tt * 128 + 128],
                            w_sb[:csz, ci, 2 * H : 3 * H],
                            start=(ci == 0),
                            stop=(ci == NCT - 1),
                        )
                    nc.vector.tensor_scalar_mul(
                        st["vp"][:, tt, :H], pv[:, :96], st["mf"][:, tt : tt + 1]
                    )

                return go

            def mk_score(b, qc, g):
                def go():
                    st = state[b]
                    ps = pss.tile([128, KG, 512], F32, tag="s", name="ps")
                    st["ps", qc, g] = ps
                    for j in range(KG):
                        kt = g * KG + j
                        nc.tensor.matmul(
                            ps[:, j, :],
                            st["k"][:, kt * 128 : kt * 128 + 128],
                            st["q"][:, qc * 512 : qc * 512 + 512],
                            start=True,
                            stop=True,
                        )

                return go

            def mk_exp(b, qc, g, on_pool=False):
                def go():
                    st = state[b]
                    es = es_pool.tile([128, KG, 512], BF16, tag="es", name="es")
                    st["es", qc, g] = es
                    ps = st.pop(("ps", qc, g))
                    if on_pool:
                        eng = nc.gpsimd
                        bias = nc.const_aps.scalar_like(0.0, ps[:])
                        eng.add_instruction(
                            mybir.InstActivation(
                                name=nc.get_next_instruction_name(),
                                func=exp,
                                ins=[
                                    eng.lower_ap(ps[:]),
                                    eng.lower_ap(bias),
                                    mybir.ImmediateValue(dtype=F32, value=SCALE),
                                    mybir.ImmediateValue(dtype=F32, value=0.0),
                                ],
                                outs=[eng.lower_ap(es[:])],
                            )
                        )
                    else:
                        nc.scalar.activation(es[:], ps[:], exp, scale=SCALE)

                return go

            def chain_part(st, b, qc, jq, lo, hi):
                """out-chain piece for one qt-tile: accumulating matmuls kt
                lo..hi-1 into this chain's dedicated PSUM bank (one open
                group per bank at a time); fin + (maybe) store at the end."""
                if ("po", qc, jq) not in st:
                    st["po", qc, jq] = pso.tile([128, 512], F32, tag="o", name="po")
                po = st["po", qc, jq]
                for kt in range(lo, hi):
                    es = st["es", qc, kt // KG]
                    nc.tensor.matmul(
                        po[:, :HP],
                        es[:, kt % KG, jq * 128 : jq * 128 + 128],
                        st["vp"][:, kt, :],
                        start=(kt == 0),
                        stop=(kt == NT - 1),
                    )
                if hi < NT:
                    return
                st.pop(("po", qc, jq))
                rec = st["rec", qc]
                ot = st["ot", qc]
                nc.vector.reciprocal(rec[:, jq : jq + 1], po[:, H : H + 1])
                nc.vector.tensor_scalar_mul(
                    ot[:, jq, :], po[:, :H], rec[:, jq : jq + 1]
                )
                if jq == NQC - 1:
                    for g in range(NG):
                        st.pop(("es", qc, g))
                    dst = out_d.ap()[b, qc * 512 : (qc + 1) * 512, :].rearrange(
                        "(p j) h -> p j h", j=NQC
                    )
                    nc.sync.dma_start(dst, st.pop(("ot", qc)))

            def mk_chain(b, qc, jq, lo=0, hi=NT):
                def go():
                    st = state[b]
                    if ("ot", qc) not in st:
                        st["rec", qc] = ot_pool.tile(
                            [128, NQC], F32, tag="rec", name="rec"
                        )
                        st["ot", qc] = ot_pool.tile(
                            [128, NQC, H], F32, tag="ot", name="ot"
                        )
                    chain_part(st, b, qc, jq, lo, hi)

                return go

            # ---- allocate persistent tiles ---------------------------------
            for b in range(BPC):
                state[b] = {}
                st = state[b]
                st["xt"] = [
                    xt_pool.tile([128, T], BF16, tag=f"xt{ci}", name=f"xt{ci}_{b}")
                    for ci in range(NCT)
                ]
                st["q"] = qk_pool.tile([96, T], BF16, tag="q", name=f"q{b}")
                st["k"] = qk_pool.tile([96, T], BF16, tag="k", name=f"k{b}")
                st["vp"] = vp_pool.tile([128, NT, HP], BF16, name=f"vp{b}")

            def c_phase(blocks, fills):
                """blocks: list of (b, qc). Chains of block i ride inside
                block i+1's score/exp stream (slots g=1,3,5,7); the final
                block's chains flush at the end."""
                pending = None
                for bi, (b, qc) in enumerate(blocks):
                    last = bi == len(blocks) - 1
                    mk_score(b, qc, 0)()
                    for g in range(NG):
                        mk_exp(b, qc, g, on_pool=(last and g in POOL_EXP_GROUPS))()
                        if g + 1 < NG:
                            mk_score(b, qc, g + 1)()
                        if pending is not None:
                            if not last and g % 2 == 1:
                                pending[g // 2]()
                            elif last and 1 <= g <= 4:
                                pending[g - 1]()
                        for u in fills.get((b, qc), [[]] * NG)[g]:
                            u()
                        if last and g == 5:
                            mk_chain(b, qc, 0, 0, NT // 2)()
                        if last and g == 6:
                            mk_chain(b, qc, 1, 0, NT // 2)()
                    if last:
                        mk_chain(b, qc, 0, NT // 2, NT)()
                        mk_chain(b, qc, 1, NT // 2, NT)()
                        mk_chain(b, qc, 2)()
                        mk_chain(b, qc, 3)()
                    else:
                        pending = [mk_chain(b, qc, jq) for jq in range(NQC)]

            def run(units):
                for u in units:
                    u()

            def P(b, nm, tc_, on_act=False):
                return mk_proj(b, nm, tc_, on_act)

            def V(b, *tts):
                return [mk_v(b, tt) for tt in tts]

            # ---- emission ---------------------------------------------------
            # DMA order: W first (needed by every matmul), then b0 x half 0
            # (kT0/qT0 critical path), mask 0, b0 x half 1, mask 1.
            # b0 x on SP/HWDGE, b1 x on GpSimd/SWDGE (parallel desc-gen).
            for ci in range(NCT):
                mk_xdma(0, 0, ci, on_pool=False)()
            mk_mask(0)()
            for ci in range(NCT):
                mk_xdma(0, 1, ci, on_pool=False)()
            mk_mask(1)()
            for half in range(2):
                for ci in range(NCT):
                    mk_xdma(1, half, ci, on_pool=False)()

            # b0 head: kT0 and qT0 matmuls interleaved per c-tile so both
            # PSUM groups fill as x arrives; copies land on ACT (k) and DVE
            # (q) in parallel. Then mask col + first v tiles for out(qc0,0).
            ppk = psx.tile([128, 512], F32, tag="p", name="ppk")[:96, :]
            ppq = psx.tile([128, 512], F32, tag="p", name="ppq")[:96, :]
            st0 = state[0]
            for ci in range(NCT):
                csz = min(128, C - ci * 128)
                for pp, off in ((ppk, H), (ppq, 0)):
                    nc.tensor.matmul(
                        pp[:, :],
                        w_sb[:csz, ci, off : off + H],
                        st0["xt"][ci][:csz, 0:512],
                        start=(ci == 0),
                        stop=(ci == NCT - 1),
                    )
            nc.scalar.copy(st0["k"][:, 0:512], ppk[:, :])
            nc.vector.tensor_copy(st0["q"][:, 0:512], ppq[:, :])
            run([mk_vcol(0)] + V(0, 0, 1))

            # Fill ledger: score(qc,g) needs kT chunk g//2 by slot g-2 and
            # qT chunk qc; out(qc,g) needs v tiles {2g,2g+1} by slot g.
            fills0 = [
                [
                    [P(0, "k", 1)],
                    V(0, 2, 3),
                    [P(0, "k", 2)] + V(0, 4, 5),
                    V(0, 6, 7),
                    [P(0, "k", 3)] + V(0, 8, 9),
                    V(0, 10, 11),
                    [P(0, "q", 1)] + V(0, 12, 13),
                    V(0, 14, 15),
                ],
                [
                    [P(0, "q", 2)],
                    [],
                    [P(1, "k", 0)],
                    [],
                    [P(1, "q", 0)],
                    [mk_vcol(1)],
                    [P(0, "q", 3)],
                    V(1, 0, 1),
                ],
                [
                    [P(1, "k", 1)],
                    V(1, 2, 3),
                    [],
                    V(1, 4, 5),
                    [],
                    [],
                    [],
                    [],
                ],
                [
                    [P(1, "k", 2)],
                    V(1, 6, 7),
                    [],
                    V(1, 8, 9),
                    [],
                    [],
                    [],
                    [],
                ],
            ]
            # b1's remaining prep rides just-in-time in its own fill
            # slots (k'3 before score g6, v' pairs before their out group,
            # q'1..3 before their qc).
            E = []
            fills1 = [
                [
                    [P(1, "k", 3)],
                    V(1, 10, 11),
                    V(1, 12, 13),
                    V(1, 14, 15),
                    [],
                    [],
                    [P(1, "q", 1)],
                    [],
                ],
                [[P(1, "q", 2)], E, E, E, E, E, E, E],
                [[P(1, "q", 3)], E, E, E, E, E, E, E],
                [E] * NG,
            ]
            fills = {}
            for qc in range(NQC):
                fills[(0, qc)] = fills0[qc]
                fills[(1, qc)] = fills1[qc]
            blocks = [(b, qc) for b in range(BPC) for qc in range(NQC)]
            c_phase(blocks, fills)

    _split_excess_waits(nc)
    return nc


_prog = None


def _get_prog():
    global _prog
    if _prog is None:
        _prog = _build()
    return _prog


def kernel(x, mask, Wk, Wq, Wv, **_ignored):
    bf16 = ml_dtypes.bfloat16
    # host-side prep: transpose x, cast to bf16, pack weights (zero-pad C)
    xt = np.ascontiguousarray(
        np.asarray(x, dtype=np.float32).transpose(0, 2, 1)
    ).astype(bf16)
    w = np.zeros((NCT * 128, 3 * H), dtype=bf16)
    w[:C] = np.concatenate(
        [np.asarray(Wq), np.asarray(Wk), np.asarray(Wv)], axis=1
    ).astype(bf16)
    maskf = np.asarray(mask).astype(np.float32).reshape(B, NT, 128).transpose(0, 2, 1)
    maskf = np.ascontiguousarray(maskf)

    nc = _get_prog()
    in_maps = [
        {
            "xt": xt[i * BPC : (i + 1) * BPC],
            "w": w,
            "maskf": maskf[i * BPC : (i + 1) * BPC],
        }
        for i in range(N_CORES)
    ]
    res = run_bass_kernel_spmd(nc, in_maps, core_ids=list(range(N_CORES)))
    raw = np.concatenate([res.results[i]["out"] for i in range(N_CORES)], axis=0)
    # un-permute the store layout: dram row qc*512 + 4p+j holds q = qc*512+128j+p
    out = raw.reshape(B, NQC, 128, 4, H).transpose(0, 1, 3, 2, 4).reshape(B, T, H)
    return np.ascontiguousarray(out)


if __name__ == "__main__":
    rng = np.random.default_rng(0)
    x = rng.standard_normal((B, T, C), dtype=np.float32)
    mask = np.ones((B, T), dtype=bool)
    s = 1.0 / np.sqrt(C)
    Wk = (rng.standard_normal((C, H)) * s).astype(np.float32)
    Wq = (rng.standard_normal((C, H)) * s).astype(np.float32)
    Wv = (rng.standard_normal((C, H)) * s).astype(np.float32)
    out = kernel(x, mask=mask, Wk=Wk, Wq=Wq, Wv=Wv)
    print("out", out.shape, out.dtype, float(np.abs(out).max()))


# revision 50
# speedup vs baseline: 1.0138x; 1.0031x over previous
"""Single-head attention (B=16, T=2048, C=576, H=96) on 8 TRN2 NeuronCores.

Sharding: data-parallel over batch — 2 batches per core; weights replicated.

All matmul operands are bf16 (fp32 PSUM accumulation); rel-err budget is
2e-2 and bf16 end-to-end measures ~7e-3. The host pre-transposes x to
xT=[C,T] bf16 and packs W=[5,128,288] (zero-padded C) bf16, so the device
never transposes anything:

  qT,kT [96,T]  = W-slice stationary, xT moving          (PSUM-accum over C)
  v_nat [T,96]  = xT-tile stationary, Wv-slice moving    (natural layout)
  vp [128,NT,98]: v*mask with mask in cols 96,97 (denominator trick)
  sT [kpos,qpos] per (qc, 2-kt group) into a 2-bank PSUM tile (bufs=2) so
     the next group's score matmuls overlap the previous group's exp
  out_nat [qpos,98] = es-slice stationary, vp moving, accumulated over kt;
     col 96 is the softmax denominator. DVE reciprocal+scale, DMA out with
     a (p j) permutation for 1536B descriptors; host un-permutes.

All x loads ride the SP/HWDGE queue in priority order (b0 half 0 first);
the tiny mask loads go through the idle GpSimd engine's SWDGE so they do
not consume SP issue slots.

This walrus build rejects >1 sync wait per instruction (and any wait on a
Drain), so after TileContext builds the module we hoist excess waits onto
injected same-engine NOPs — semantics identical since engines execute
their stream in order.
"""

import sys

if "/opt/trn_rl_repo" not in sys.path:
    sys.path.insert(0, "/opt/trn_rl_repo")

import ml_dtypes
import numpy as np

import concourse.bass as bass
import concourse.tile as tile
from concourse import mybir
from concourse.bass_utils import run_bass_kernel_spmd

N_CORES = 8
B, T, C, H = 16, 2048, 576, 96
BPC = B // N_CORES  # batches per core
SCALE = 1.0 / float(np.sqrt(H))

F32 = mybir.dt.float32
BF16 = mybir.dt.bfloat16

NT = T // 128  # 16 key tiles
NCT = (C + 127) // 128  # 5 c-tiles (last is 64)
NQC = T // 512  # 4 query chunks
NG = 8  # kt-groups per query chunk (2 kt each)
KG = NT // NG  # 2 kt per group
HP = H + 2  # 98: H + denominator col + dup (even moving count)
POOL_EXP_GROUPS = ()  # which kt-groups' exp runs on GpSimd instead of ACT


def _split_excess_waits(nc, max_waits=1):
    """Hoist sync waits beyond this walrus's per-instruction limit onto
    injected NOPs that run just before, on the same engine."""
    n_split = 0
    for fn in nc.m.functions:
        for blk in fn.blocks:
            new_insts = []
            changed = False
            for inst in blk.instructions:
                si = inst.sync_info
                waits = list(si.on_wait) if si is not None else []
                cap = 0 if isinstance(inst, mybir.InstDrain) else max_waits
                if len(waits) > cap:
                    # Keep the most meaningful wait ON the instruction (its
                    # engine-stage wait doesn't block the sequencer); push
                    # self-engine sems (trivially satisfied in-order) and
                    # DMA-completion WARs onto the NOPs, which DO block SEQ.
                    eng = str(inst.engine).split(".")[-1].split(":")[0].strip("'\" >")

                    def prio(iw):
                        i, w = iw
                        nm = getattr(w, "ant_name", "") or ""
                        self_sem = nm.startswith(eng)
                        dma_sem = nm.startswith("DMAHW") or nm.startswith("DMASW")
                        return (0 if self_sem else (1 if dma_sem else 2), i)

                    order = sorted(enumerate(waits), key=prio)
                    waits = [w for _, w in order]
                    excess = waits[:-cap] if cap else waits
                    keep = waits[-cap:] if cap else []
                    for i in range(0, len(excess), max_waits):
                        chunk = excess[i : i + max_waits]
                        new_insts.append(
                            mybir.InstNoOp(
                                name=f"{inst.name}-wsplit{i}",
                                engine=inst.engine,
                                ins=[],
                                outs=[],
                                sync_info=mybir.SyncInfo(on_wait=chunk, on_update=[]),
                            )
                        )
                    inst.sync_info = mybir.SyncInfo(
                        on_wait=keep, on_update=list(si.on_update)
                    )
                    changed = True
                    n_split += 1
                new_insts.append(inst)
            if changed:
                blk.instructions = new_insts
    return n_split


def _build():
    nc = bass.Bass("TRN2", target_bir_lowering=False, debug=False)

    xt_d = nc.dram_tensor("xt", [BPC, C, T], BF16, kind="ExternalInput")
    w_d = nc.dram_tensor("w", [NCT * 128, 3 * H], BF16, kind="ExternalInput")
    mf_d = nc.dram_tensor("maskf", [BPC, 128, NT], F32, kind="ExternalInput")
    out_d = nc.dram_tensor("out", [BPC, T, H], F32, kind="ExternalOutput")

    exp = mybir.ActivationFunctionType.Exp

    with tile.TileContext(nc) as tc:
        with (
            tc.tile_pool(name="const", bufs=1) as const_pool,
            tc.tile_pool(name="xt", bufs=2) as xt_pool,
            tc.tile_pool(name="qk", bufs=2) as qk_pool,
            tc.tile_pool(name="vp", bufs=2) as vp_pool,
            tc.tile_pool(name="mk", bufs=2) as mk_pool,
            tc.tile_pool(name="es", bufs=17) as es_pool,
            tc.tile_pool(name="ot", bufs=4) as ot_pool,
            tc.tile_pool(name="psx", bufs=2, space="PSUM") as psx,  # proj+v
            tc.tile_pool(name="pso", bufs=2, space="PSUM") as pso,  # out chains
            tc.tile_pool(name="pss", bufs=2, space="PSUM") as pss,  # scores 2-bank
        ):
            # PE p-state warm-up: dependency-free dummy matmuls on garbage
            # SBUF data while the head DMAs land (PE is idle anyway). After
            # ~3us of continuous busy the cost model (and hardware) runs the
            # array at full clock, so the first real matmuls aren't 2x slow.
            # The psum bank is reset by its next user's start=True matmul.
            dmy = const_pool.tile([128, 640], BF16, name="dmy")
            nc.vector.memset(dmy[:], 0)
            dpo = pso.tile([128, 512], F32, tag="o", name="dpo")
            for _ in range(1):
                nc.tensor.matmul(
                    dpo[:, :],
                    dmy[:, 0:128],
                    dmy[:, 128:640],
                    start=True,
                    stop=True,
                )

            w_sb = const_pool.tile([128, NCT, 3 * H], BF16, name="w_sb")
            nc.sync.dma_start(
                w_sb[:], w_d.ap().rearrange("(g p) c -> p g c", p=128)
            )
            # pre-warm the exp table so the first real exp doesn't pay the
            # table load inside the pipeline
            warm = const_pool.tile([128, 2], F32, name="warm")
            nc.scalar.activation(warm[:], w_sb[:, 0, 0:4].bitcast(F32), exp)

            state = {}

            def mk_mask(b):
                def go():
                    mf = mk_pool.tile([128, NT], F32, name=f"mf{b}")
                    state[b]["mf"] = mf
                    nc.gpsimd.dma_start(mf[:], mf_d.ap()[b])

                return go

            def mk_xdma(b, half, ci, on_pool):
                def go():
                    st = state[b]
                    csz = min(128, C - ci * 128)
                    lo = half * 1024
                    if on_pool:
                        # GpSimd/SWDGE piece: overlap one column with the SP
                        # half-0 piece of the same tile (rewritten with the
                        # same data) so the WAW dependency queues this
                        # transfer BEHIND the critical half-0 stream instead
                        # of stealing its DMA-engine slots.
                        lo -= 1
                    dst = st["xt"][ci][:csz, lo : half * 1024 + 1024]
                    src = xt_d.ap()[
                        b, ci * 128 : ci * 128 + csz, lo : half * 1024 + 1024
                    ]
                    if on_pool:
                        nc.gpsimd.dma_start(dst, src)
                    else:
                        nc.sync.dma_start(dst, src)

                return go

            def mk_proj(b, nm, tc_, on_act):
                """qT/kT chunk: stationary W slice, moving xT; PSUM accum over C."""
                off = 0 if nm == "q" else H

                def go():
                    st = state[b]
                    dst = st[nm]
                    pp = psx.tile([128, 512], F32, tag="p", name="pp")[:96, :]
                    for ci in range(NCT):
                        csz = min(128, C - ci * 128)
                        nc.tensor.matmul(
                            pp[:, :],
                            w_sb[:csz, ci, off : off + H],
                            st["xt"][ci][:csz, tc_ * 512 : tc_ * 512 + 512],
                            start=(ci == 0),
                            stop=(ci == NCT - 1),
                        )
                    if nm == "k":
                        # split the copy so the first half unblocks its
                        # score group before the second half drains
                        for h2 in range(2):
                            cdst = dst[:, tc_ * 512 + h2 * 256 : tc_ * 512 + h2 * 256 + 256]
                            csrc = pp[:, h2 * 256 : h2 * 256 + 256]
                            if on_act:
                                nc.scalar.copy(cdst, csrc)
                            else:
                                nc.vector.tensor_copy(cdst, csrc)
                    else:
                        cdst = dst[:, tc_ * 512 : tc_ * 512 + 512]
                        if on_act:
                            nc.scalar.copy(cdst, pp[:, :])
                        else:
                            nc.vector.tensor_copy(cdst, pp[:, :])

                return go

            def mk_vcol(b):
                def go():
                    st = state[b]
                    vp, mf = st["vp"], st["mf"]
                    src = mf[:].rearrange("p (k o) -> p k o", o=1)
                    nc.vector.tensor_copy(vp[:, :, H : H + 1], src)
                    nc.vector.tensor_copy(vp[:, :, H + 1 : H + 2], src)

                return go

            def mk_v(b, tt):
                """v natural tile: stationary xT slice, moving Wv slice."""

                def go():
                    st = state[b]
                    pv = psx.tile([128, 512], F32, tag="p", name="pv")
                    for ci in range(NCT):
                        csz = min(128, C - ci * 128)
                        nc.tensor.matmul(
                            pv[:, :96],
                            st["xt"][ci][:csz, tt * 128 : tt * 128 + 128],
                            w_sb[:csz, ci, 2 * H : 3 * H],
                            start=(ci == 0),
                            stop=(ci == NCT - 1),
                        )
                    nc.vector.tensor_scalar_mul(
                        st["vp"][:, tt, :H], pv[:, :96], st["mf"][:, tt : tt + 1]
                    )

                return go

            def mk_score(b, qc, g):
                def go():
                    st = state[b]
                    ps = pss.tile([128, KG, 512], F32, tag="s", name="ps")
                    st["ps", qc, g] = ps
                    for j in range(KG):
                        kt = g * KG + j
                        nc.tensor.matmul(
                            ps[:, j, :],
                            st["k"][:, kt * 128 : kt * 128 + 128],
                            st["q"][:, qc * 512 : qc * 512 + 512],
                            start=True,
                            stop=True,
                        )

                return go

            def mk_exp(b, qc, g, on_pool=False):
                def go():
                    st = state[b]
                    es = es_pool.tile([128, KG, 512], BF16, tag="es", name="es")
                    st["es", qc, g] = es
                    ps = st.pop(("ps", qc, g))
                    if on_pool:
                        eng = nc.gpsimd
                        bias = nc.const_aps.scalar_like(0.0, ps[:])
                        eng.add_instruction(
                            mybir.InstActivation(
                                name=nc.get_next_instruction_name(),
                                func=exp,
                                ins=[
                                    eng.lower_ap(ps[:]),
                                    eng.lower_ap(bias),
                                    mybir.ImmediateValue(dtype=F32, value=SCALE),
                                    mybir.ImmediateValue(dtype=F32, value=0.0),
                                ],
                                outs=[eng.lower_ap(es[:])],
                            )
                        )
                    else:
                        nc.scalar.activation(es[:], ps[:], exp, scale=SCALE)

                return go

            def chain_part(st, b, qc, jq, lo, hi, split_store=False):
                """out-chain piece for one qt-tile: accumulating matmuls kt
                lo..hi-1 into this chain's dedicated PSUM bank (one open
                group per bank at a time); fin + (maybe) store at the end."""
                if ("po", qc, jq) not in st:
                    st["po", qc, jq] = pso.tile([128, 512], F32, tag="o", name="po")
                po = st["po", qc, jq]
                for kt in range(lo, hi):
                    es = st["es", qc, kt // KG]
                    nc.tensor.matmul(
                        po[:, :HP],
                        es[:, kt % KG, jq * 128 : jq * 128 + 128],
                        st["vp"][:, kt, :],
                        start=(kt == 0),
                        stop=(kt == NT - 1),
                    )
                if hi < NT:
                    return
                st.pop(("po", qc, jq))
                rec = st["rec", qc]
                ot = st["ot", qc]
                nc.vector.reciprocal(rec[:, jq : jq + 1], po[:, H : H + 1])
                nc.vector.tensor_scalar_mul(
                    ot[:, jq, :], po[:, :H], rec[:, jq : jq + 1]
                )
                if split_store and jq == NQC - 2:
                    # final block: ship qt-tiles 0-2 early so only a small
                    # quarter store sits on the critical tail
                    dst = out_d.ap()[b, qc * 512 : (qc + 1) * 512, :].rearrange(
                        "(p j) h -> p j h", j=NQC
                    )[:, 0 : NQC - 1, :]
                    nc.sync.dma_start(dst, st["ot", qc][:, 0 : NQC - 1, :])
                if jq == NQC - 1:
                    for g in range(NG):
                        st.pop(("es", qc, g))
                    dst = out_d.ap()[b, qc * 512 : (qc + 1) * 512, :].rearrange(
                        "(p j) h -> p j h", j=NQC
                    )
                    ot_t = st.pop(("ot", qc))
                    if split_store:
                        nc.sync.dma_start(
                            dst[:, NQC - 1 : NQC, :], ot_t[:, NQC - 1 : NQC, :]
                        )
                    else:
                        nc.sync.dma_start(dst, ot_t)

            def mk_chain(b, qc, jq, lo=0, hi=NT, split_store=False):
                def go():
                    st = state[b]
                    if ("ot", qc) not in st:
                        st["rec", qc] = ot_pool.tile(
                            [128, NQC], F32, tag="rec", name="rec"
                        )
                        st["ot", qc] = ot_pool.tile(
                            [128, NQC, H], F32, tag="ot", name="ot"
                        )
                    chain_part(st, b, qc, jq, lo, hi, split_store)

                return go

            # ---- allocate persistent tiles ---------------------------------
            for b in range(BPC):
                state[b] = {}
                st = state[b]
                st["xt"] = [
                    xt_pool.tile([128, T], BF16, tag=f"xt{ci}", name=f"xt{ci}_{b}")
                    for ci in range(NCT)
                ]
                st["q"] = qk_pool.tile([96, T], BF16, tag="q", name=f"q{b}")
                st["k"] = qk_pool.tile([96, T], BF16, tag="k", name=f"k{b}")
                st["vp"] = vp_pool.tile([128, NT, HP], BF16, name=f"vp{b}")

            def c_phase(blocks, fills):
                """blocks: list of (b, qc). Chains of block i ride inside
                block i+1's score/exp stream (slots g=1,3,5,7); the final
                block's chains flush at the end."""
                pending = None
                for bi, (b, qc) in enumerate(blocks):
                    last = bi == len(blocks) - 1
                    mk_score(b, qc, 0)()
                    for g in range(NG):
                        mk_exp(b, qc, g, on_pool=(last and g in POOL_EXP_GROUPS))()
                        if g + 1 < NG:
                            mk_score(b, qc, g + 1)()
                        if pending is not None:
                            if not last and g % 2 == 1:
                                pending[g // 2]()
                            elif last and 1 <= g <= 4:
                                pending[g - 1]()
                        for u in fills.get((b, qc), [[]] * NG)[g]:
                            u()
                        if last and g == 5:
                            mk_chain(b, qc, 0, 0, NT // 2)()
                        if last and g == 6:
                            mk_chain(b, qc, 1, 0, NT // 2)()
                    if last:
                        mk_chain(b, qc, 0, NT // 2, NT)()
                        mk_chain(b, qc, 1, NT // 2, NT)()
                        mk_chain(b, qc, 2, split_store=True)()
                        mk_chain(b, qc, 3, split_store=True)()
                    else:
                        pending = [mk_chain(b, qc, jq) for jq in range(NQC)]

            def run(units):
                for u in units:
                    u()

            def P(b, nm, tc_, on_act=False):
                return mk_proj(b, nm, tc_, on_act)

            def V(b, *tts):
                return [mk_v(b, tt) for tt in tts]

            # ---- emission ---------------------------------------------------
            # DMA order: W first (needed by every matmul), then b0 x half 0
            # (kT0/qT0 critical path), mask 0, b0 x half 1, mask 1.
            # b0 x on SP/HWDGE, b1 x on GpSimd/SWDGE (parallel desc-gen).
            for ci in range(NCT):
                mk_xdma(0, 0, ci, on_pool=False)()
            mk_mask(0)()
            for ci in range(NCT):
                mk_xdma(0, 1, ci, on_pool=False)()
            mk_mask(1)()
            for half in range(2):
                for ci in range(NCT):
                    mk_xdma(1, half, ci, on_pool=False)()

            # b0 head: kT0 and qT0 matmuls interleaved per c-tile so both
            # PSUM groups fill as x arrives; copies land on ACT (k) and DVE
            # (q) in parallel. Then mask col + first v tiles for out(qc0,0).
            ppk = psx.tile([128, 512], F32, tag="p", name="ppk")[:96, :]
            ppq = psx.tile([128, 512], F32, tag="p", name="ppq")[:96, :]
            st0 = state[0]
            for ci in range(NCT):
                csz = min(128, C - ci * 128)
                for pp, off in ((ppk, H), (ppq, 0)):
                    nc.tensor.matmul(
                        pp[:, :],
                        w_sb[:csz, ci, off : off + H],
                        st0["xt"][ci][:csz, 0:512],
                        start=(ci == 0),
                        stop=(ci == NCT - 1),
                    )
            nc.scalar.copy(st0["k"][:, 0:512], ppk[:, :])
            nc.vector.tensor_copy(st0["q"][:, 0:512], ppq[:, :])
            run([mk_vcol(0)] + V(0, 0, 1))

            # Fill ledger: score(qc,g) needs kT chunk g//2 by slot g-2 and
            # qT chunk qc; out(qc,g) needs v tiles {2g,2g+1} by slot g.
            fills0 = [
                [
                    [P(0, "k", 1)],
                    V(0, 2, 3),
                    [P(0, "k", 2)] + V(0, 4, 5),
                    V(0, 6, 7),
                    [P(0, "k", 3)] + V(0, 8, 9),
                    V(0, 10, 11),
                    [P(0, "q", 1)] + V(0, 12, 13),
                    V(0, 14, 15),
                ],
                [
                    [P(0, "q", 2)],
                    [],
                    [P(1, "k", 0)],
                    [],
                    [P(1, "q", 0)],
                    [mk_vcol(1)],
                    [P(0, "q", 3)],
                    V(1, 0, 1),
                ],
                [
                    [P(1, "k", 1)],
                    V(1, 2, 3),
                    [],
                    V(1, 4, 5),
                    [],
                    [],
                    [],
                    [],
                ],
                [
                    [P(1, "k", 2)],
                    V(1, 6, 7),
                    [],
                    V(1, 8, 9),
                    [],
                    [],
                    [],
                    [],
                ],
            ]
            # b1's remaining prep rides just-in-time in its own fill
            # slots (k'3 before score g6, v' pairs before their out group,
            # q'1..3 before their qc).
            E = []
            fills1 = [
                [
                    [P(1, "k", 3)],
                    V(1, 10, 11),
                    V(1, 12, 13),
                    V(1, 14, 15),
                    [],
                    [],
                    [P(1, "q", 1)],
                    [],
                ],
                [[P(1, "q", 2)], E, E, E, E, E, E, E],
                [[P(1, "q", 3)], E, E, E, E, E, E, E],
                [E] * NG,
            ]
            fills = {}
            for qc in range(NQC):
                fills[(0, qc)] = fills0[qc]
                fills[(1, qc)] = fills1[qc]
            blocks = [(b, qc) for b in range(BPC) for qc in range(NQC)]
            c_phase(blocks, fills)

    _split_excess_waits(nc)
    return nc


_prog = None


def _get_prog():
    global _prog
    if _prog is None:
        _prog = _build()
    return _prog


def kernel(x, mask, Wk, Wq, Wv, **_ignored):
    bf16 = ml_dtypes.bfloat16
    # host-side prep: transpose x, cast to bf16, pack weights (zero-pad C)
    xt = np.ascontiguousarray(
        np.asarray(x, dtype=np.float32).transpose(0, 2, 1)
    ).astype(bf16)
    w = np.zeros((NCT * 128, 3 * H), dtype=bf16)
    w[:C] = np.concatenate(
        [np.asarray(Wq), np.asarray(Wk), np.asarray(Wv)], axis=1
    ).astype(bf16)
    maskf = np.asarray(mask).astype(np.float32).reshape(B, NT, 128).transpose(0, 2, 1)
    maskf = np.ascontiguousarray(maskf)

    nc = _get_prog()
    in_maps = [
        {
            "xt": xt[i * BPC : (i + 1) * BPC],
            "w": w,
            "maskf": maskf[i * BPC : (i + 1) * BPC],
        }
        for i in range(N_CORES)
    ]
    res = run_bass_kernel_spmd(nc, in_maps, core_ids=list(range(N_CORES)))
    raw = np.concatenate([res.results[i]["out"] for i in range(N_CORES)], axis=0)
    # un-permute the store layout: dram row qc*512 + 4p+j holds q = qc*512+128j+p
    out = raw.reshape(B, NQC, 128, 4, H).transpose(0, 1, 3, 2, 4).reshape(B, T, H)
    return np.ascontiguousarray(out)


if __name__ == "__main__":
    rng = np.random.default_rng(0)
    x = rng.standard_normal((B, T, C), dtype=np.float32)
    mask = np.ones((B, T), dtype=bool)
    s = 1.0 / np.sqrt(C)
    Wk = (rng.standard_normal((C, H)) * s).astype(np.float32)
    Wq = (rng.standard_normal((C, H)) * s).astype(np.float32)
    Wv = (rng.standard_normal((C, H)) * s).astype(np.float32)
    out = kernel(x, mask=mask, Wk=Wk, Wq=Wq, Wv=Wv)
    print("out", out.shape, out.dtype, float(np.abs(out).max()))


# revision 52
# speedup vs baseline: 1.0228x; 1.0089x over previous
"""Single-head attention (B=16, T=2048, C=576, H=96) on 8 TRN2 NeuronCores.

Sharding: data-parallel over batch — 2 batches per core; weights replicated.

All matmul operands are bf16 (fp32 PSUM accumulation); rel-err budget is
2e-2 and bf16 end-to-end measures ~7e-3. The host pre-transposes x to
xT=[C,T] bf16 and packs W=[5,128,288] (zero-padded C) bf16, so the device
never transposes anything:

  qT,kT [96,T]  = W-slice stationary, xT moving          (PSUM-accum over C)
  v_nat [T,96]  = xT-tile stationary, Wv-slice moving    (natural layout)
  vp [128,NT,98]: v*mask with mask in cols 96,97 (denominator trick)
  sT [kpos,qpos] per (qc, 2-kt group) into a 2-bank PSUM tile (bufs=2) so
     the next group's score matmuls overlap the previous group's exp
  out_nat [qpos,98] = es-slice stationary, vp moving, accumulated over kt;
     col 96 is the softmax denominator. DVE reciprocal+scale, DMA out with
     a (p j) permutation for 1536B descriptors; host un-permutes.

All x loads ride the SP/HWDGE queue in priority order (b0 half 0 first);
the tiny mask loads go through the idle GpSimd engine's SWDGE so they do
not consume SP issue slots.

This walrus build rejects >1 sync wait per instruction (and any wait on a
Drain), so after TileContext builds the module we hoist excess waits onto
injected same-engine NOPs — semantics identical since engines execute
their stream in order.
"""

import sys

if "/opt/trn_rl_repo" not in sys.path:
    sys.path.insert(0, "/opt/trn_rl_repo")

import ml_dtypes
import numpy as np

import concourse.bass as bass
import concourse.tile as tile
from concourse import mybir
from concourse.bass_utils import run_bass_kernel_spmd

N_CORES = 8
B, T, C, H = 16, 2048, 576, 96
BPC = B // N_CORES  # batches per core
SCALE = 1.0 / float(np.sqrt(H))

F32 = mybir.dt.float32
BF16 = mybir.dt.bfloat16

NT = T // 128  # 16 key tiles
NCT = (C + 127) // 128  # 5 c-tiles (last is 64)
NQC = T // 512  # 4 query chunks
NG = 8  # kt-groups per query chunk (2 kt each)
KG = NT // NG  # 2 kt per group
HP = H + 2  # 98: H + denominator col + dup (even moving count)
POOL_EXP_GROUPS = ()  # which kt-groups' exp runs on GpSimd instead of ACT


def _split_excess_waits(nc, max_waits=1):
    """Hoist sync waits beyond this walrus's per-instruction limit onto
    injected NOPs that run just before, on the same engine."""
    n_split = 0
    for fn in nc.m.functions:
        for blk in fn.blocks:
            new_insts = []
            changed = False
            for inst in blk.instructions:
                si = inst.sync_info
                waits = list(si.on_wait) if si is not None else []
                cap = 0 if isinstance(inst, mybir.InstDrain) else max_waits
                if len(waits) > cap:
                    # Keep the most meaningful wait ON the instruction (its
                    # engine-stage wait doesn't block the sequencer); push
                    # self-engine sems (trivially satisfied in-order) and
                    # DMA-completion WARs onto the NOPs, which DO block SEQ.
                    eng = str(inst.engine).split(".")[-1].split(":")[0].strip("'\" >")

                    def prio(iw):
                        i, w = iw
                        nm = getattr(w, "ant_name", "") or ""
                        self_sem = nm.startswith(eng)
                        dma_sem = nm.startswith("DMAHW") or nm.startswith("DMASW")
                        return (0 if self_sem else (1 if dma_sem else 2), i)

                    order = sorted(enumerate(waits), key=prio)
                    waits = [w for _, w in order]
                    excess = waits[:-cap] if cap else waits
                    keep = waits[-cap:] if cap else []
                    for i in range(0, len(excess), max_waits):
                        chunk = excess[i : i + max_waits]
                        new_insts.append(
                            mybir.InstNoOp(
                                name=f"{inst.name}-wsplit{i}",
                                engine=inst.engine,
                                ins=[],
                                outs=[],
                                sync_info=mybir.SyncInfo(on_wait=chunk, on_update=[]),
                            )
                        )
                    inst.sync_info = mybir.SyncInfo(
                        on_wait=keep, on_update=list(si.on_update)
                    )
                    changed = True
                    n_split += 1
                new_insts.append(inst)
            if changed:
                blk.instructions = new_insts
    return n_split


def _build():
    nc = bass.Bass("TRN2", target_bir_lowering=False, debug=False)

    xt_d = nc.dram_tensor("xt", [BPC, C, T], BF16, kind="ExternalInput")
    w_d = nc.dram_tensor("w", [NCT * 128, 3 * H], BF16, kind="ExternalInput")
    mf_d = nc.dram_tensor("maskf", [BPC, 128, NT], F32, kind="ExternalInput")
    out_d = nc.dram_tensor("out", [BPC, T, H], F32, kind="ExternalOutput")

    exp = mybir.ActivationFunctionType.Exp

    with tile.TileContext(nc) as tc:
        with (
            tc.tile_pool(name="const", bufs=1) as const_pool,
            tc.tile_pool(name="xt", bufs=2) as xt_pool,
            tc.tile_pool(name="qk", bufs=2) as qk_pool,
            tc.tile_pool(name="vp", bufs=2) as vp_pool,
            tc.tile_pool(name="mk", bufs=2) as mk_pool,
            tc.tile_pool(name="es", bufs=17) as es_pool,
            tc.tile_pool(name="ot", bufs=4) as ot_pool,
            tc.tile_pool(name="psx", bufs=2, space="PSUM") as psx,  # proj+v
            tc.tile_pool(name="pso", bufs=2, space="PSUM") as pso,  # out chains
            tc.tile_pool(name="pss", bufs=2, space="PSUM") as pss,  # scores 2-bank
        ):
            # PE p-state warm-up: dependency-free dummy matmuls on garbage
            # SBUF data while the head DMAs land (PE is idle anyway). After
            # ~3us of continuous busy the cost model (and hardware) runs the
            # array at full clock, so the first real matmuls aren't 2x slow.
            # The psum bank is reset by its next user's start=True matmul.
            dmy = const_pool.tile([128, 640], BF16, name="dmy")
            nc.vector.memset(dmy[:], 0)
            dpo = pso.tile([128, 512], F32, tag="o", name="dpo")
            for _ in range(1):
                nc.tensor.matmul(
                    dpo[:, :],
                    dmy[:, 0:128],
                    dmy[:, 128:640],
                    start=True,
                    stop=True,
                )

            w_sb = const_pool.tile([128, NCT, 3 * H], BF16, name="w_sb")
            nc.sync.dma_start(
                w_sb[:], w_d.ap().rearrange("(g p) c -> p g c", p=128)
            )
            # pre-warm the exp table so the first real exp doesn't pay the
            # table load inside the pipeline
            warm = const_pool.tile([128, 2], F32, name="warm")
            nc.scalar.activation(warm[:], w_sb[:, 0, 0:4].bitcast(F32), exp)

            state = {}

            def mk_mask(b):
                def go():
                    mf = mk_pool.tile([128, NT], F32, name=f"mf{b}")
                    state[b]["mf"] = mf
                    nc.gpsimd.dma_start(mf[:], mf_d.ap()[b])

                return go

            def mk_xdma(b, half, ci, on_pool):
                def go():
                    st = state[b]
                    csz = min(128, C - ci * 128)
                    lo = half * 1024
                    if on_pool:
                        # GpSimd/SWDGE piece: overlap one column with the SP
                        # half-0 piece of the same tile (rewritten with the
                        # same data) so the WAW dependency queues this
                        # transfer BEHIND the critical half-0 stream instead
                        # of stealing its DMA-engine slots.
                        lo -= 1
                    dst = st["xt"][ci][:csz, lo : half * 1024 + 1024]
                    src = xt_d.ap()[
                        b, ci * 128 : ci * 128 + csz, lo : half * 1024 + 1024
                    ]
                    if on_pool:
                        nc.gpsimd.dma_start(dst, src)
                    else:
                        nc.sync.dma_start(dst, src)

                return go

            def mk_proj(b, nm, tc_, on_act):
                """qT/kT chunk: stationary W slice, moving xT; PSUM accum over C."""
                off = 0 if nm == "q" else H

                def go():
                    st = state[b]
                    dst = st[nm]
                    pp = psx.tile([128, 512], F32, tag="p", name="pp")[:96, :]
                    for ci in range(NCT):
                        csz = min(128, C - ci * 128)
                        nc.tensor.matmul(
                            pp[:, :],
                            w_sb[:csz, ci, off : off + H],
                            st["xt"][ci][:csz, tc_ * 512 : tc_ * 512 + 512],
                            start=(ci == 0),
                            stop=(ci == NCT - 1),
                        )
                    if nm == "k":
                        # split the copy so the first half unblocks its
                        # score group before the second half drains
                        for h2 in range(2):
                            cdst = dst[:, tc_ * 512 + h2 * 256 : tc_ * 512 + h2 * 256 + 256]
                            csrc = pp[:, h2 * 256 : h2 * 256 + 256]
                            if on_act:
                                nc.scalar.copy(cdst, csrc)
                            else:
                                nc.vector.tensor_copy(cdst, csrc)
                    else:
                        cdst = dst[:, tc_ * 512 : tc_ * 512 + 512]
                        if on_act:
                            nc.scalar.copy(cdst, pp[:, :])
                        else:
                            nc.vector.tensor_copy(cdst, pp[:, :])

                return go

            def mk_vcol(b):
                def go():
                    st = state[b]
                    vp, mf = st["vp"], st["mf"]
                    src = mf[:].rearrange("p (k o) -> p k o", o=1)
                    nc.vector.tensor_copy(vp[:, :, H : H + 1], src)
                    nc.vector.tensor_copy(vp[:, :, H + 1 : H + 2], src)

                return go

            def mk_v(b, tt):
                """v natural tile: stationary xT slice, moving Wv slice."""

                def go():
                    st = state[b]
                    pv = psx.tile([128, 512], F32, tag="p", name="pv")
                    for ci in range(NCT):
                        csz = min(128, C - ci * 128)
                        nc.tensor.matmul(
                            pv[:, :96],
                            st["xt"][ci][:csz, tt * 128 : tt * 128 + 128],
                            w_sb[:csz, ci, 2 * H : 3 * H],
                            start=(ci == 0),
                            stop=(ci == NCT - 1),
                        )
                    nc.vector.tensor_scalar_mul(
                        st["vp"][:, tt, :H], pv[:, :96], st["mf"][:, tt : tt + 1]
                    )

                return go

            def mk_score(b, qc, g):
                def go():
                    st = state[b]
                    ps = pss.tile([128, KG, 512], F32, tag="s", name="ps")
                    st["ps", qc, g] = ps
                    for j in range(KG):
                        kt = g * KG + j
                        nc.tensor.matmul(
                            ps[:, j, :],
                            st["k"][:, kt * 128 : kt * 128 + 128],
                            st["q"][:, qc * 512 : qc * 512 + 512],
                            start=True,
                            stop=True,
                        )

                return go

            def mk_exp(b, qc, g, on_pool=False):
                def go():
                    st = state[b]
                    es = es_pool.tile([128, KG, 512], BF16, tag="es", name="es")
                    st["es", qc, g] = es
                    ps = st.pop(("ps", qc, g))
                    if on_pool:
                        eng = nc.gpsimd
                        bias = nc.const_aps.scalar_like(0.0, ps[:])
                        eng.add_instruction(
                            mybir.InstActivation(
                                name=nc.get_next_instruction_name(),
                                func=exp,
                                ins=[
                                    eng.lower_ap(ps[:]),
                                    eng.lower_ap(bias),
                                    mybir.ImmediateValue(dtype=F32, value=SCALE),
                                    mybir.ImmediateValue(dtype=F32, value=0.0),
                                ],
                                outs=[eng.lower_ap(es[:])],
                            )
                        )
                    else:
                        nc.scalar.activation(es[:], ps[:], exp, scale=SCALE)

                return go

            def chain_part(st, b, qc, jq, lo, hi, split_store=False, alt_pool=False):
                """out-chain piece for one qt-tile: accumulating matmuls kt
                lo..hi-1 into this chain's dedicated PSUM bank (one open
                group per bank at a time); fin + (maybe) store at the end.
                alt_pool borrows the proj/v pool's banks (idle in the final
                blocks) so four chains can be open at once."""
                if ("po", qc, jq) not in st:
                    if alt_pool:
                        st["po", qc, jq] = psx.tile([128, 512], F32, tag="p", name="po")
                    else:
                        st["po", qc, jq] = pso.tile([128, 512], F32, tag="o", name="po")
                po = st["po", qc, jq]
                for kt in range(lo, hi):
                    es = st["es", qc, kt // KG]
                    nc.tensor.matmul(
                        po[:, :HP],
                        es[:, kt % KG, jq * 128 : jq * 128 + 128],
                        st["vp"][:, kt, :],
                        start=(kt == 0),
                        stop=(kt == NT - 1),
                    )
                if hi < NT:
                    return
                st.pop(("po", qc, jq))
                rec = st["rec", qc]
                ot = st["ot", qc]
                nc.vector.reciprocal(rec[:, jq : jq + 1], po[:, H : H + 1])
                nc.vector.tensor_scalar_mul(
                    ot[:, jq, :], po[:, :H], rec[:, jq : jq + 1]
                )
                if split_store and jq == NQC - 2:
                    # final block: ship qt-tiles 0-2 early so only a small
                    # quarter store sits on the critical tail
                    dst = out_d.ap()[b, qc * 512 : (qc + 1) * 512, :].rearrange(
                        "(p j) h -> p j h", j=NQC
                    )[:, 0 : NQC - 1, :]
                    nc.sync.dma_start(dst, st["ot", qc][:, 0 : NQC - 1, :])
                if jq == NQC - 1:
                    for g in range(NG):
                        st.pop(("es", qc, g))
                    dst = out_d.ap()[b, qc * 512 : (qc + 1) * 512, :].rearrange(
                        "(p j) h -> p j h", j=NQC
                    )
                    ot_t = st.pop(("ot", qc))
                    if split_store:
                        nc.sync.dma_start(
                            dst[:, NQC - 1 : NQC, :], ot_t[:, NQC - 1 : NQC, :]
                        )
                    else:
                        nc.sync.dma_start(dst, ot_t)

            def mk_chain(b, qc, jq, lo=0, hi=NT, split_store=False, alt_pool=False):
                def go():
                    st = state[b]
                    if ("ot", qc) not in st:
                        st["rec", qc] = ot_pool.tile(
                            [128, NQC], F32, tag="rec", name="rec"
                        )
                        st["ot", qc] = ot_pool.tile(
                            [128, NQC, H], F32, tag="ot", name="ot"
                        )
                    chain_part(st, b, qc, jq, lo, hi, split_store, alt_pool)

                return go

            # ---- allocate persistent tiles ---------------------------------
            for b in range(BPC):
                state[b] = {}
                st = state[b]
                st["xt"] = [
                    xt_pool.tile([128, T], BF16, tag=f"xt{ci}", name=f"xt{ci}_{b}")
                    for ci in range(NCT)
                ]
                st["q"] = qk_pool.tile([96, T], BF16, tag="q", name=f"q{b}")
                st["k"] = qk_pool.tile([96, T], BF16, tag="k", name=f"k{b}")
                st["vp"] = vp_pool.tile([128, NT, HP], BF16, name=f"vp{b}")

            def c_phase(blocks, fills):
                """blocks: list of (b, qc). Chains of block i ride inside
                block i+1's score/exp stream (slots g=1,3,5,7); the final
                block's chains flush at the end."""
                pending = None
                for bi, (b, qc) in enumerate(blocks):
                    last = bi == len(blocks) - 1
                    mk_score(b, qc, 0)()
                    for g in range(NG):
                        mk_exp(b, qc, g, on_pool=(last and g in POOL_EXP_GROUPS))()
                        if g + 1 < NG:
                            mk_score(b, qc, g + 1)()
                        if pending is not None:
                            if not last and g % 2 == 1:
                                pending[g // 2]()
                            elif last and g <= 3:
                                pending[g]()
                        for u in fills.get((b, qc), [[]] * NG)[g]:
                            u()
                        if last and g == 4:
                            mk_chain(b, qc, 0, 0, NT // 2)()
                        if last and g == 5:
                            mk_chain(b, qc, 1, 0, NT // 2)()
                        if last and g == 6:
                            mk_chain(b, qc, 2, 0, NT // 2, alt_pool=True)()
                        if last and g == 7:
                            mk_chain(b, qc, 3, 0, NT // 2, alt_pool=True)()
                    if last:
                        mk_chain(b, qc, 0, NT // 2, NT)()
                        mk_chain(b, qc, 1, NT // 2, NT)()
                        mk_chain(b, qc, 2, NT // 2, NT, split_store=True, alt_pool=True)()
                        mk_chain(b, qc, 3, NT // 2, NT, split_store=True, alt_pool=True)()
                    else:
                        pending = [mk_chain(b, qc, jq) for jq in range(NQC)]

            def run(units):
                for u in units:
                    u()

            def P(b, nm, tc_, on_act=False):
                return mk_proj(b, nm, tc_, on_act)

            def V(b, *tts):
                return [mk_v(b, tt) for tt in tts]

            # ---- emission ---------------------------------------------------
            # DMA order: W first (needed by every matmul), then b0 x half 0
            # (kT0/qT0 critical path), mask 0, b0 x half 1, mask 1.
            # b0 x on SP/HWDGE, b1 x on GpSimd/SWDGE (parallel desc-gen).
            for ci in range(NCT):
                mk_xdma(0, 0, ci, on_pool=False)()
            mk_mask(0)()
            for ci in range(NCT):
                mk_xdma(0, 1, ci, on_pool=False)()
            mk_mask(1)()
            for half in range(2):
                for ci in range(NCT):
                    mk_xdma(1, half, ci, on_pool=False)()

            # b0 head: kT0 and qT0 matmuls interleaved per c-tile so both
            # PSUM groups fill as x arrives; copies land on ACT (k) and DVE
            # (q) in parallel. Then mask col + first v tiles for out(qc0,0).
            ppk = psx.tile([128, 512], F32, tag="p", name="ppk")[:96, :]
            ppq = psx.tile([128, 512], F32, tag="p", name="ppq")[:96, :]
            st0 = state[0]
            for ci in range(NCT):
                csz = min(128, C - ci * 128)
                for pp, off in ((ppk, H), (ppq, 0)):
                    nc.tensor.matmul(
                        pp[:, :],
                        w_sb[:csz, ci, off : off + H],
                        st0["xt"][ci][:csz, 0:512],
                        start=(ci == 0),
                        stop=(ci == NCT - 1),
                    )
            nc.scalar.copy(st0["k"][:, 0:512], ppk[:, :])
            nc.vector.tensor_copy(st0["q"][:, 0:512], ppq[:, :])
            run([mk_vcol(0)] + V(0, 0, 1))

            # Fill ledger: score(qc,g) needs kT chunk g//2 by slot g-2 and
            # qT chunk qc; out(qc,g) needs v tiles {2g,2g+1} by slot g.
            fills0 = [
                [
                    [P(0, "k", 1)],
                    V(0, 2, 3),
                    [P(0, "k", 2)] + V(0, 4, 5),
                    V(0, 6, 7),
                    [P(0, "k", 3)] + V(0, 8, 9),
                    V(0, 10, 11),
                    [P(0, "q", 1)] + V(0, 12, 13),
                    V(0, 14, 15),
                ],
                [
                    [P(0, "q", 2)],
                    [],
                    [P(1, "k", 0)],
                    [],
                    [P(1, "q", 0)],
                    [mk_vcol(1)],
                    [P(0, "q", 3)],
                    V(1, 0, 1),
                ],
                [
                    [P(1, "k", 1)],
                    V(1, 2, 3),
                    [],
                    V(1, 4, 5),
                    [],
                    [],
                    [],
                    [],
                ],
                [
                    [P(1, "k", 2)],
                    V(1, 6, 7),
                    [],
                    V(1, 8, 9),
                    [],
                    [],
                    [],
                    [],
                ],
            ]
            # b1's remaining prep rides just-in-time in its own fill
            # slots (k'3 before score g6, v' pairs before their out group,
            # q'1..3 before their qc).
            E = []
            fills1 = [
                [
                    [P(1, "k", 3)],
                    V(1, 10, 11),
                    V(1, 12, 13),
                    V(1, 14, 15),
                    [],
                    [],
                    [P(1, "q", 1)],
                    [],
                ],
                [[P(1, "q", 2)], E, E, E, E, E, E, E],
                [[P(1, "q", 3)], E, E, E, E, E, E, E],
                [E] * NG,
            ]
            fills = {}
            for qc in range(NQC):
                fills[(0, qc)] = fills0[qc]
                fills[(1, qc)] = fills1[qc]
            blocks = [(b, qc) for b in range(BPC) for qc in range(NQC)]
            c_phase(blocks, fills)

    _split_excess_waits(nc)
    return nc


_prog = None


def _get_prog():
    global _prog
    if _prog is None:
        _prog = _build()
    return _prog


def kernel(x, mask, Wk, Wq, Wv, **_ignored):
    bf16 = ml_dtypes.bfloat16
    # host-side prep: transpose x, cast to bf16, pack weights (zero-pad C)
    xt = np.ascontiguousarray(
        np.asarray(x, dtype=np.float32).transpose(0, 2, 1)
    ).astype(bf16)
    w = np.zeros((NCT * 128, 3 * H), dtype=bf16)
    w[:C] = np.concatenate(
        [np.asarray(Wq), np.asarray(Wk), np.asarray(Wv)], axis=1
    ).astype(bf16)
    maskf = np.asarray(mask).astype(np.float32).reshape(B, NT, 128).transpose(0, 2, 1)
    maskf = np.ascontiguousarray(maskf)

    nc = _get_prog()
    in_maps = [
        {
            "xt": xt[i * BPC : (i + 1) * BPC],
            "w": w,
            "maskf": maskf[i * BPC : (i + 1) * BPC],
        }
        for i in range(N_CORES)
    ]
    res = run_bass_kernel_spmd(nc, in_maps, core_ids=list(range(N_CORES)))
    raw = np.concatenate([res.results[i]["out"] for i in range(N_CORES)], axis=0)
    # un-permute the store layout: dram row qc*512 + 4p+j holds q = qc*512+128j+p
    out = raw.reshape(B, NQC, 128, 4, H).transpose(0, 1, 3, 2, 4).reshape(B, T, H)
    return np.ascontiguousarray(out)


if __name__ == "__main__":
    rng = np.random.default_rng(0)
    x = rng.standard_normal((B, T, C), dtype=np.float32)
    mask = np.ones((B, T), dtype=bool)
    s = 1.0 / np.sqrt(C)
    Wk = (rng.standard_normal((C, H)) * s).astype(np.float32)
    Wq = (rng.standard_normal((C, H)) * s).astype(np.float32)
    Wv = (rng.standard_normal((C, H)) * s).astype(np.float32)
    out = kernel(x, mask=mask, Wk=Wk, Wq=Wq, Wv=Wv)
    print("out", out.shape, out.dtype, float(np.abs(out).max()))


# revision 53
# speedup vs baseline: 1.0255x; 1.0026x over previous
"""Single-head attention (B=16, T=2048, C=576, H=96) on 8 TRN2 NeuronCores.

Sharding: data-parallel over batch — 2 batches per core; weights replicated.

All matmul operands are bf16 (fp32 PSUM accumulation); rel-err budget is
2e-2 and bf16 end-to-end measures ~7e-3. The host pre-transposes x to
xT=[C,T] bf16 and packs W=[5,128,288] (zero-padded C) bf16, so the device
never transposes anything:

  qT,kT [96,T]  = W-slice stationary, xT moving          (PSUM-accum over C)
  v_nat [T,96]  = xT-tile stationary, Wv-slice moving    (natural layout)
  vp [128,NT,98]: v*mask with mask in cols 96,97 (denominator trick)
  sT [kpos,qpos] per (qc, 2-kt group) into a 2-bank PSUM tile (bufs=2) so
     the next group's score matmuls overlap the previous group's exp
  out_nat [qpos,98] = es-slice stationary, vp moving, accumulated over kt;
     col 96 is the softmax denominator. DVE reciprocal+scale, DMA out with
     a (p j) permutation for 1536B descriptors; host un-permutes.

All x loads ride the SP/HWDGE queue in priority order (b0 half 0 first);
the tiny mask loads go through the idle GpSimd engine's SWDGE so they do
not consume SP issue slots.

This walrus build rejects >1 sync wait per instruction (and any wait on a
Drain), so after TileContext builds the module we hoist excess waits onto
injected same-engine NOPs — semantics identical since engines execute
their stream in order.
"""

import sys

if "/opt/trn_rl_repo" not in sys.path:
    sys.path.insert(0, "/opt/trn_rl_repo")

import ml_dtypes
import numpy as np

import concourse.bass as bass
import concourse.tile as tile
from concourse import mybir
from concourse.bass_utils import run_bass_kernel_spmd

N_CORES = 8
B, T, C, H = 16, 2048, 576, 96
BPC = B // N_CORES  # batches per core
SCALE = 1.0 / float(np.sqrt(H))

F32 = mybir.dt.float32
BF16 = mybir.dt.bfloat16

NT = T // 128  # 16 key tiles
NCT = (C + 127) // 128  # 5 c-tiles (last is 64)
NQC = T // 512  # 4 query chunks
NG = 8  # kt-groups per query chunk (2 kt each)
KG = NT // NG  # 2 kt per group
HP = H + 2  # 98: H + denominator col + dup (even moving count)
POOL_EXP_GROUPS = ()  # which kt-groups' exp runs on GpSimd instead of ACT


def _split_excess_waits(nc, max_waits=1):
    """Hoist sync waits beyond this walrus's per-instruction limit onto
    injected NOPs that run just before, on the same engine."""
    n_split = 0
    for fn in nc.m.functions:
        for blk in fn.blocks:
            new_insts = []
            changed = False
            for inst in blk.instructions:
                si = inst.sync_info
                waits = list(si.on_wait) if si is not None else []
                cap = 0 if isinstance(inst, mybir.InstDrain) else max_waits
                if len(waits) > cap:
                    # Keep the most meaningful wait ON the instruction (its
                    # engine-stage wait doesn't block the sequencer); push
                    # self-engine sems (trivially satisfied in-order) and
                    # DMA-completion WARs onto the NOPs, which DO block SEQ.
                    eng = str(inst.engine).split(".")[-1].split(":")[0].strip("'\" >")

                    def prio(iw):
                        i, w = iw
                        nm = getattr(w, "ant_name", "") or ""
                        self_sem = nm.startswith(eng)
                        dma_sem = nm.startswith("DMAHW") or nm.startswith("DMASW")
                        return (0 if self_sem else (1 if dma_sem else 2), i)

                    order = sorted(enumerate(waits), key=prio)
                    waits = [w for _, w in order]
                    excess = waits[:-cap] if cap else waits
                    keep = waits[-cap:] if cap else []
                    for i in range(0, len(excess), max_waits):
                        chunk = excess[i : i + max_waits]
                        new_insts.append(
                            mybir.InstNoOp(
                                name=f"{inst.name}-wsplit{i}",
                                engine=inst.engine,
                                ins=[],
                                outs=[],
                                sync_info=mybir.SyncInfo(on_wait=chunk, on_update=[]),
                            )
                        )
                    inst.sync_info = mybir.SyncInfo(
                        on_wait=keep, on_update=list(si.on_update)
                    )
                    changed = True
                    n_split += 1
                new_insts.append(inst)
            if changed:
                blk.instructions = new_insts
    return n_split


def _build():
    nc = bass.Bass("TRN2", target_bir_lowering=False, debug=False)

    xt_d = nc.dram_tensor("xt", [BPC, C, T], BF16, kind="ExternalInput")
    w_d = nc.dram_tensor("w", [NCT * 128, 3 * H], BF16, kind="ExternalInput")
    mf_d = nc.dram_tensor("maskf", [BPC, 128, NT], F32, kind="ExternalInput")
    out_d = nc.dram_tensor("out", [BPC, T, H], F32, kind="ExternalOutput")

    exp = mybir.ActivationFunctionType.Exp

    with tile.TileContext(nc) as tc:
        with (
            tc.tile_pool(name="const", bufs=1) as const_pool,
            tc.tile_pool(name="xt", bufs=2) as xt_pool,
            tc.tile_pool(name="qk", bufs=2) as qk_pool,
            tc.tile_pool(name="vp", bufs=2) as vp_pool,
            tc.tile_pool(name="mk", bufs=2) as mk_pool,
            tc.tile_pool(name="es", bufs=17) as es_pool,
            tc.tile_pool(name="ot", bufs=4) as ot_pool,
            tc.tile_pool(name="psx", bufs=2, space="PSUM") as psx,  # proj+v
            tc.tile_pool(name="pso", bufs=2, space="PSUM") as pso,  # out chains
            tc.tile_pool(name="pss", bufs=2, space="PSUM") as pss,  # scores 2-bank
        ):
            # PE p-state warm-up: dependency-free dummy matmuls on garbage
            # SBUF data while the head DMAs land (PE is idle anyway). After
            # ~3us of continuous busy the cost model (and hardware) runs the
            # array at full clock, so the first real matmuls aren't 2x slow.
            # The psum bank is reset by its next user's start=True matmul.
            dmy = const_pool.tile([128, 640], BF16, name="dmy")
            nc.vector.memset(dmy[:], 0)
            dpo = pso.tile([128, 512], F32, tag="o", name="dpo")
            for _ in range(1):
                nc.tensor.matmul(
                    dpo[:, :],
                    dmy[:, 0:128],
                    dmy[:, 128:640],
                    start=True,
                    stop=True,
                )

            w_sb = const_pool.tile([128, NCT, 3 * H], BF16, name="w_sb")
            nc.sync.dma_start(
                w_sb[:], w_d.ap().rearrange("(g p) c -> p g c", p=128)
            )
            # pre-warm the exp table so the first real exp doesn't pay the
            # table load inside the pipeline
            warm = const_pool.tile([128, 2], F32, name="warm")
            nc.scalar.activation(warm[:], w_sb[:, 0, 0:4].bitcast(F32), exp)

            state = {}

            def mk_mask(b):
                def go():
                    mf = mk_pool.tile([128, NT], F32, name=f"mf{b}")
                    state[b]["mf"] = mf
                    nc.gpsimd.dma_start(mf[:], mf_d.ap()[b])

                return go

            def mk_xdma(b, half, ci, on_pool):
                def go():
                    st = state[b]
                    csz = min(128, C - ci * 128)
                    lo = half * 1024
                    if on_pool:
                        # GpSimd/SWDGE piece: overlap one column with the SP
                        # half-0 piece of the same tile (rewritten with the
                        # same data) so the WAW dependency queues this
                        # transfer BEHIND the critical half-0 stream instead
                        # of stealing its DMA-engine slots.
                        lo -= 1
                    dst = st["xt"][ci][:csz, lo : half * 1024 + 1024]
                    src = xt_d.ap()[
                        b, ci * 128 : ci * 128 + csz, lo : half * 1024 + 1024
                    ]
                    if on_pool:
                        nc.gpsimd.dma_start(dst, src)
                    else:
                        nc.sync.dma_start(dst, src)

                return go

            def mk_proj(b, nm, tc_, on_act):
                """qT/kT chunk: stationary W slice, moving xT; PSUM accum over C."""
                off = 0 if nm == "q" else H

                def go():
                    st = state[b]
                    dst = st[nm]
                    pp = psx.tile([128, 512], F32, tag="p", name="pp")[:96, :]
                    for ci in range(NCT):
                        csz = min(128, C - ci * 128)
                        nc.tensor.matmul(
                            pp[:, :],
                            w_sb[:csz, ci, off : off + H],
                            st["xt"][ci][:csz, tc_ * 512 : tc_ * 512 + 512],
                            start=(ci == 0),
                            stop=(ci == NCT - 1),
                        )
                    if nm == "k":
                        # split the copy so the first half unblocks its
                        # score group before the second half drains
                        for h2 in range(2):
                            cdst = dst[:, tc_ * 512 + h2 * 256 : tc_ * 512 + h2 * 256 + 256]
                            csrc = pp[:, h2 * 256 : h2 * 256 + 256]
                            if on_act:
                                nc.scalar.copy(cdst, csrc)
                            else:
                                nc.vector.tensor_copy(cdst, csrc)
                    else:
                        cdst = dst[:, tc_ * 512 : tc_ * 512 + 512]
                        if on_act:
                            nc.scalar.copy(cdst, pp[:, :])
                        else:
                            nc.vector.tensor_copy(cdst, pp[:, :])

                return go

            def mk_vcol(b):
                def go():
                    st = state[b]
                    vp, mf = st["vp"], st["mf"]
                    src = mf[:].rearrange("p (k o) -> p k o", o=1)
                    nc.vector.tensor_copy(vp[:, :, H : H + 1], src)
                    nc.vector.tensor_copy(vp[:, :, H + 1 : H + 2], src)

                return go

            def mk_v(b, tt):
                """v natural tile: stationary xT slice, moving Wv slice."""

                def go():
                    st = state[b]
                    pv = psx.tile([128, 512], F32, tag="p", name="pv")
                    for ci in range(NCT):
                        csz = min(128, C - ci * 128)
                        nc.tensor.matmul(
                            pv[:, :96],
                            st["xt"][ci][:csz, tt * 128 : tt * 128 + 128],
                            w_sb[:csz, ci, 2 * H : 3 * H],
                            start=(ci == 0),
                            stop=(ci == NCT - 1),
                        )
                    nc.vector.tensor_scalar_mul(
                        st["vp"][:, tt, :H], pv[:, :96], st["mf"][:, tt : tt + 1]
                    )

                return go

            def mk_score(b, qc, g):
                def go():
                    st = state[b]
                    ps = pss.tile([128, KG, 512], F32, tag="s", name="ps")
                    st["ps", qc, g] = ps
                    for j in range(KG):
                        kt = g * KG + j
                        nc.tensor.matmul(
                            ps[:, j, :],
                            st["k"][:, kt * 128 : kt * 128 + 128],
                            st["q"][:, qc * 512 : qc * 512 + 512],
                            start=True,
                            stop=True,
                        )

                return go

            def mk_exp(b, qc, g, on_pool=False):
                def go():
                    st = state[b]
                    es = es_pool.tile([128, KG, 512], BF16, tag="es", name="es")
                    st["es", qc, g] = es
                    ps = st.pop(("ps", qc, g))
                    if on_pool:
                        eng = nc.gpsimd
                        bias = nc.const_aps.scalar_like(0.0, ps[:])
                        eng.add_instruction(
                            mybir.InstActivation(
                                name=nc.get_next_instruction_name(),
                                func=exp,
                                ins=[
                                    eng.lower_ap(ps[:]),
                                    eng.lower_ap(bias),
                                    mybir.ImmediateValue(dtype=F32, value=SCALE),
                                    mybir.ImmediateValue(dtype=F32, value=0.0),
                                ],
                                outs=[eng.lower_ap(es[:])],
                            )
                        )
                    else:
                        nc.scalar.activation(es[:], ps[:], exp, scale=SCALE)

                return go

            def chain_part(st, b, qc, jq, lo, hi, split_store=False, alt_pool=False):
                """out-chain piece for one qt-tile: accumulating matmuls kt
                lo..hi-1 into this chain's dedicated PSUM bank (one open
                group per bank at a time); fin + (maybe) store at the end.
                alt_pool borrows the proj/v pool's banks (idle in the final
                blocks) so four chains can be open at once."""
                if ("po", qc, jq) not in st:
                    if alt_pool:
                        st["po", qc, jq] = psx.tile([128, 512], F32, tag="p", name="po")
                    else:
                        st["po", qc, jq] = pso.tile([128, 512], F32, tag="o", name="po")
                po = st["po", qc, jq]
                for kt in range(lo, hi):
                    es = st["es", qc, kt // KG]
                    nc.tensor.matmul(
                        po[:, :HP],
                        es[:, kt % KG, jq * 128 : jq * 128 + 128],
                        st["vp"][:, kt, :],
                        start=(kt == 0),
                        stop=(kt == NT - 1),
                    )
                if hi < NT:
                    return
                st.pop(("po", qc, jq))
                rec = st["rec", qc]
                ot = st["ot", qc]
                nc.vector.reciprocal(rec[:, jq : jq + 1], po[:, H : H + 1])
                if split_store and jq % 2 == 1:
                    nc.scalar.mul(ot[:, jq, :], po[:, :H], rec[:, jq : jq + 1])
                else:
                    nc.vector.tensor_scalar_mul(
                        ot[:, jq, :], po[:, :H], rec[:, jq : jq + 1]
                    )
                if split_store and jq == NQC - 2:
                    # final block: ship qt-tiles 0-2 early so only a small
                    # quarter store sits on the critical tail
                    dst = out_d.ap()[b, qc * 512 : (qc + 1) * 512, :].rearrange(
                        "(p j) h -> p j h", j=NQC
                    )[:, 0 : NQC - 1, :]
                    nc.sync.dma_start(dst, st["ot", qc][:, 0 : NQC - 1, :])
                if jq == NQC - 1:
                    for g in range(NG):
                        st.pop(("es", qc, g))
                    dst = out_d.ap()[b, qc * 512 : (qc + 1) * 512, :].rearrange(
                        "(p j) h -> p j h", j=NQC
                    )
                    ot_t = st.pop(("ot", qc))
                    if split_store:
                        nc.sync.dma_start(
                            dst[:, NQC - 1 : NQC, :], ot_t[:, NQC - 1 : NQC, :]
                        )
                    else:
                        nc.sync.dma_start(dst, ot_t)

            def mk_chain(b, qc, jq, lo=0, hi=NT, split_store=False, alt_pool=False):
                def go():
                    st = state[b]
                    if ("ot", qc) not in st:
                        st["rec", qc] = ot_pool.tile(
                            [128, NQC], F32, tag="rec", name="rec"
                        )
                        st["ot", qc] = ot_pool.tile(
                            [128, NQC, H], F32, tag="ot", name="ot"
                        )
                    chain_part(st, b, qc, jq, lo, hi, split_store, alt_pool)

                return go

            # ---- allocate persistent tiles ---------------------------------
            for b in range(BPC):
                state[b] = {}
                st = state[b]
                st["xt"] = [
                    xt_pool.tile([128, T], BF16, tag=f"xt{ci}", name=f"xt{ci}_{b}")
                    for ci in range(NCT)
                ]
                st["q"] = qk_pool.tile([96, T], BF16, tag="q", name=f"q{b}")
                st["k"] = qk_pool.tile([96, T], BF16, tag="k", name=f"k{b}")
                st["vp"] = vp_pool.tile([128, NT, HP], BF16, name=f"vp{b}")

            def c_phase(blocks, fills):
                """blocks: list of (b, qc). Chains of block i ride inside
                block i+1's score/exp stream (slots g=1,3,5,7); the final
                block's chains flush at the end."""
                pending = None
                for bi, (b, qc) in enumerate(blocks):
                    last = bi == len(blocks) - 1
                    mk_score(b, qc, 0)()
                    for g in range(NG):
                        mk_exp(b, qc, g, on_pool=(last and g in POOL_EXP_GROUPS))()
                        if g + 1 < NG:
                            mk_score(b, qc, g + 1)()
                        if pending is not None:
                            if not last and g % 2 == 1:
                                pending[g // 2]()
                            elif last and g <= 3:
                                pending[g]()
                        for u in fills.get((b, qc), [[]] * NG)[g]:
                            u()
                        if last and g == 4:
                            mk_chain(b, qc, 0, 0, NT // 2)()
                        if last and g == 5:
                            mk_chain(b, qc, 1, 0, NT // 2)()
                        if last and g == 6:
                            mk_chain(b, qc, 2, 0, NT // 2, alt_pool=True)()
                            mk_chain(b, qc, 0, NT // 2, NT - KG)()
                        if last and g == 7:
                            mk_chain(b, qc, 3, 0, NT // 2, alt_pool=True)()
                            mk_chain(b, qc, 1, NT // 2, NT - KG)()
                            mk_chain(b, qc, 2, NT // 2, NT - KG, alt_pool=True)()
                            mk_chain(b, qc, 3, NT // 2, NT - KG, alt_pool=True)()
                    if last:
                        mk_chain(b, qc, 0, NT - KG, NT, split_store=True)()
                        mk_chain(b, qc, 1, NT - KG, NT, split_store=True)()
                        mk_chain(b, qc, 2, NT - KG, NT, split_store=True, alt_pool=True)()
                        mk_chain(b, qc, 3, NT - KG, NT, split_store=True, alt_pool=True)()
                    else:
                        pending = [mk_chain(b, qc, jq) for jq in range(NQC)]

            def run(units):
                for u in units:
                    u()

            def P(b, nm, tc_, on_act=False):
                return mk_proj(b, nm, tc_, on_act)

            def V(b, *tts):
                return [mk_v(b, tt) for tt in tts]

            # ---- emission ---------------------------------------------------
            # DMA order: W first (needed by every matmul), then b0 x half 0
            # (kT0/qT0 critical path), mask 0, b0 x half 1, mask 1.
            # b0 x on SP/HWDGE, b1 x on GpSimd/SWDGE (parallel desc-gen).
            for ci in range(NCT):
                mk_xdma(0, 0, ci, on_pool=False)()
            mk_mask(0)()
            for ci in range(NCT):
                mk_xdma(0, 1, ci, on_pool=False)()
            mk_mask(1)()
            for half in range(2):
                for ci in range(NCT):
                    mk_xdma(1, half, ci, on_pool=False)()

            # b0 head: kT0 and qT0 matmuls interleaved per c-tile so both
            # PSUM groups fill as x arrives; copies land on ACT (k) and DVE
            # (q) in parallel. Then mask col + first v tiles for out(qc0,0).
            ppk = psx.tile([128, 512], F32, tag="p", name="ppk")[:96, :]
            ppq = psx.tile([128, 512], F32, tag="p", name="ppq")[:96, :]
            st0 = state[0]
            for ci in range(NCT):
                csz = min(128, C - ci * 128)
                for pp, off in ((ppk, H), (ppq, 0)):
                    nc.tensor.matmul(
                        pp[:, :],
                        w_sb[:csz, ci, off : off + H],
                        st0["xt"][ci][:csz, 0:512],
                        start=(ci == 0),
                        stop=(ci == NCT - 1),
                    )
            nc.scalar.copy(st0["k"][:, 0:512], ppk[:, :])
            nc.vector.tensor_copy(st0["q"][:, 0:512], ppq[:, :])
            run([mk_vcol(0)] + V(0, 0, 1))

            # Fill ledger: score(qc,g) needs kT chunk g//2 by slot g-2 and
            # qT chunk qc; out(qc,g) needs v tiles {2g,2g+1} by slot g.
            fills0 = [
                [
                    [P(0, "k", 1)],
                    V(0, 2, 3),
                    [P(0, "k", 2)] + V(0, 4, 5),
                    V(0, 6, 7),
                    [P(0, "k", 3)] + V(0, 8, 9),
                    V(0, 10, 11),
                    [P(0, "q", 1)] + V(0, 12, 13),
                    V(0, 14, 15),
                ],
                [
                    [P(0, "q", 2)],
                    [],
                    [P(1, "k", 0)],
                    [],
                    [P(1, "q", 0)],
                    [mk_vcol(1)],
                    [P(0, "q", 3)],
                    V(1, 0, 1),
                ],
                [
                    [P(1, "k", 1)],
                    V(1, 2, 3),
                    [],
                    V(1, 4, 5),
                    [],
                    [],
                    [],
                    [],
                ],
                [
                    [P(1, "k", 2)],
                    V(1, 6, 7),
                    [],
                    V(1, 8, 9),
                    [],
                    [],
                    [],
                    [],
                ],
            ]
            # b1's remaining prep rides just-in-time in its own fill
            # slots (k'3 before score g6, v' pairs before their out group,
            # q'1..3 before their qc).
            E = []
            fills1 = [
                [
                    [P(1, "k", 3)],
                    V(1, 10, 11),
                    V(1, 12, 13),
                    V(1, 14, 15),
                    [],
                    [],
                    [P(1, "q", 1)],
                    [],
                ],
                [[P(1, "q", 2)], E, E, E, E, E, E, E],
                [[P(1, "q", 3)], E, E, E, E, E, E, E],
                [E] * NG,
            ]
            fills = {}
            for qc in range(NQC):
                fills[(0, qc)] = fills0[qc]
                fills[(1, qc)] = fills1[qc]
            blocks = [(b, qc) for b in range(BPC) for qc in range(NQC)]
            c_phase(blocks, fills)

    _split_excess_waits(nc)
    return nc


_prog = None


def _get_prog():
    global _prog
    if _prog is None:
        _prog = _build()
    return _prog


def kernel(x, mask, Wk, Wq, Wv, **_ignored):
    bf16 = ml_dtypes.bfloat16
    # host-side prep: transpose x, cast to bf16, pack weights (zero-pad C)
    xt = np.ascontiguousarray(
        np.asarray(x, dtype=np.float32).transpose(0, 2, 1)
    ).astype(bf16)
    w = np.zeros((NCT * 128, 3 * H), dtype=bf16)
    w[:C] = np.concatenate(
        [np.asarray(Wq), np.asarray(Wk), np.asarray(Wv)], axis=1
    ).astype(bf16)
    maskf = np.asarray(mask).astype(np.float32).reshape(B, NT, 128).transpose(0, 2, 1)
    maskf = np.ascontiguousarray(maskf)

    nc = _get_prog()
    in_maps = [
        {
            "xt": xt[i * BPC : (i + 1) * BPC],
            "w": w,
            "maskf": maskf[i * BPC : (i + 1) * BPC],
        }
        for i in range(N_CORES)
    ]
    res = run_bass_kernel_spmd(nc, in_maps, core_ids=list(range(N_CORES)))
    raw = np.concatenate([res.results[i]["out"] for i in range(N_CORES)], axis=0)
    # un-permute the store layout: dram row qc*512 + 4p+j holds q = qc*512+128j+p
    out = raw.reshape(B, NQC, 128, 4, H).transpose(0, 1, 3, 2, 4).reshape(B, T, H)
    return np.ascontiguousarray(out)


if __name__ == "__main__":
    rng = np.random.default_rng(0)
    x = rng.standard_normal((B, T, C), dtype=np.float32)
    mask = np.ones((B, T), dtype=bool)
    s = 1.0 / np.sqrt(C)
    Wk = (rng.standard_normal((C, H)) * s).astype(np.float32)
    Wq = (rng.standard_normal((C, H)) * s).astype(np.float32)
    Wv = (rng.standard_normal((C, H)) * s).astype(np.float32)
    out = kernel(x, mask=mask, Wk=Wk, Wq=Wq, Wv=Wv)
    print("out", out.shape, out.dtype, float(np.abs(out).max()))


# revision 58
# speedup vs baseline: 1.0263x; 1.0008x over previous
"""Single-head attention (B=16, T=2048, C=576, H=96) on 8 TRN2 NeuronCores.

Sharding: data-parallel over batch — 2 batches per core; weights replicated.

All matmul operands are bf16 (fp32 PSUM accumulation); rel-err budget is
2e-2 and bf16 end-to-end measures ~7e-3. The host pre-transposes x to
xT=[C,T] bf16 and packs W=[5,128,288] (zero-padded C) bf16, so the device
never transposes anything:

  qT,kT [96,T]  = W-slice stationary, xT moving          (PSUM-accum over C)
  v_nat [T,96]  = xT-tile stationary, Wv-slice moving    (natural layout)
  vp [128,NT,98]: v*mask with mask in cols 96,97 (denominator trick)
  sT [kpos,qpos] per (qc, 2-kt group) into a 2-bank PSUM tile (bufs=2) so
     the next group's score matmuls overlap the previous group's exp
  out_nat [qpos,98] = es-slice stationary, vp moving, accumulated over kt;
     col 96 is the softmax denominator. DVE reciprocal+scale, DMA out with
     a (p j) permutation for 1536B descriptors; host un-permutes.

All x loads ride the SP/HWDGE queue in priority order (b0 half 0 first);
the tiny mask loads go through the idle GpSimd engine's SWDGE so they do
not consume SP issue slots.

This walrus build rejects >1 sync wait per instruction (and any wait on a
Drain), so after TileContext builds the module we hoist excess waits onto
injected same-engine NOPs — semantics identical since engines execute
their stream in order.
"""

import sys

if "/opt/trn_rl_repo" not in sys.path:
    sys.path.insert(0, "/opt/trn_rl_repo")

import ml_dtypes
import numpy as np

import concourse.bass as bass
import concourse.tile as tile
from concourse import mybir
from concourse.bass_utils import run_bass_kernel_spmd

N_CORES = 8
B, T, C, H = 16, 2048, 576, 96
BPC = B // N_CORES  # batches per core
SCALE = 1.0 / float(np.sqrt(H))

F32 = mybir.dt.float32
BF16 = mybir.dt.bfloat16

NT = T // 128  # 16 key tiles
NCT = (C + 127) // 128  # 5 c-tiles (last is 64)
NQC = T // 512  # 4 query chunks
NG = 8  # kt-groups per query chunk (2 kt each)
KG = NT // NG  # 2 kt per group
HP = H + 2  # 98: H + denominator col + dup (even moving count)
POOL_EXP_GROUPS = ()  # which kt-groups' exp runs on GpSimd instead of ACT


def _split_excess_waits(nc, max_waits=1):
    """Hoist sync waits beyond this walrus's per-instruction limit onto
    injected NOPs that run just before, on the same engine."""
    n_split = 0
    for fn in nc.m.functions:
        for blk in fn.blocks:
            new_insts = []
            changed = False
            for inst in blk.instructions:
                si = inst.sync_info
                waits = list(si.on_wait) if si is not None else []
                cap = 0 if isinstance(inst, mybir.InstDrain) else max_waits
                if len(waits) > cap:
                    # Keep the most meaningful wait ON the instruction (its
                    # engine-stage wait doesn't block the sequencer); push
                    # self-engine sems (trivially satisfied in-order) and
                    # DMA-completion WARs onto the NOPs, which DO block SEQ.
                    eng = str(inst.engine).split(".")[-1].split(":")[0].strip("'\" >")

                    def prio(iw):
                        i, w = iw
                        nm = getattr(w, "ant_name", "") or ""
                        self_sem = nm.startswith(eng)
                        dma_sem = nm.startswith("DMAHW") or nm.startswith("DMASW")
                        return (0 if self_sem else (1 if dma_sem else 2), i)

                    order = sorted(enumerate(waits), key=prio)
                    waits = [w for _, w in order]
                    excess = waits[:-cap] if cap else waits
                    keep = waits[-cap:] if cap else []
                    for i in range(0, len(excess), max_waits):
                        chunk = excess[i : i + max_waits]
                        new_insts.append(
                            mybir.InstNoOp(
                                name=f"{inst.name}-wsplit{i}",
                                engine=inst.engine,
                                ins=[],
                                outs=[],
                                sync_info=mybir.SyncInfo(on_wait=chunk, on_update=[]),
                            )
                        )
                    inst.sync_info = mybir.SyncInfo(
                        on_wait=keep, on_update=list(si.on_update)
                    )
                    changed = True
                    n_split += 1
                new_insts.append(inst)
            if changed:
                blk.instructions = new_insts
    return n_split


def _build():
    nc = bass.Bass("TRN2", target_bir_lowering=False, debug=False)

    xt_d = nc.dram_tensor("xt", [BPC, C, T], BF16, kind="ExternalInput")
    w_d = nc.dram_tensor("w", [NCT * 128, 3 * H], BF16, kind="ExternalInput")
    mf_d = nc.dram_tensor("maskf", [BPC, 128, NT], F32, kind="ExternalInput")
    out_d = nc.dram_tensor("out", [BPC, T, H], F32, kind="ExternalOutput")

    exp = mybir.ActivationFunctionType.Exp

    with tile.TileContext(nc) as tc:
        with (
            tc.tile_pool(name="const", bufs=1) as const_pool,
            tc.tile_pool(name="xt", bufs=2) as xt_pool,
            tc.tile_pool(name="qk", bufs=2) as qk_pool,
            tc.tile_pool(name="vp", bufs=2) as vp_pool,
            tc.tile_pool(name="mk", bufs=2) as mk_pool,
            tc.tile_pool(name="es", bufs=18) as es_pool,
            tc.tile_pool(name="ot", bufs=4) as ot_pool,
            tc.tile_pool(name="psx", bufs=2, space="PSUM") as psx,  # proj+v
            tc.tile_pool(name="pso", bufs=2, space="PSUM") as pso,  # out chains
            tc.tile_pool(name="pss", bufs=2, space="PSUM") as pss,  # scores 2-bank
        ):
            # PE p-state warm-up: dependency-free dummy matmuls on garbage
            # SBUF data while the head DMAs land (PE is idle anyway). After
            # ~3us of continuous busy the cost model (and hardware) runs the
            # array at full clock, so the first real matmuls aren't 2x slow.
            # The psum bank is reset by its next user's start=True matmul.
            dmy = const_pool.tile([128, 640], BF16, name="dmy")
            nc.vector.memset(dmy[:], 0)
            dpo = pso.tile([128, 512], F32, tag="o", name="dpo")
            for _ in range(1):
                nc.tensor.matmul(
                    dpo[:, :],
                    dmy[:, 0:128],
                    dmy[:, 128:640],
                    start=True,
                    stop=True,
                )

            w_sb = const_pool.tile([128, NCT, 3 * H], BF16, name="w_sb")
            nc.sync.dma_start(
                w_sb[:], w_d.ap().rearrange("(g p) c -> p g c", p=128)
            )
            # pre-warm the exp table so the first real exp doesn't pay the
            # table load inside the pipeline
            warm = const_pool.tile([128, 2], F32, name="warm")
            nc.scalar.activation(warm[:], w_sb[:, 0, 0:4].bitcast(F32), exp)

            state = {}

            def mk_mask(b):
                def go():
                    mf = mk_pool.tile([128, NT], F32, name=f"mf{b}")
                    state[b]["mf"] = mf
                    nc.gpsimd.dma_start(mf[:], mf_d.ap()[b])

                return go

            def mk_xdma(b, half, ci, on_pool):
                def go():
                    st = state[b]
                    csz = min(128, C - ci * 128)
                    lo = half * 1024
                    if on_pool:
                        # GpSimd/SWDGE piece: overlap one column with the SP
                        # half-0 piece of the same tile (rewritten with the
                        # same data) so the WAW dependency queues this
                        # transfer BEHIND the critical half-0 stream instead
                        # of stealing its DMA-engine slots.
                        lo -= 1
                    dst = st["xt"][ci][:csz, lo : half * 1024 + 1024]
                    src = xt_d.ap()[
                        b, ci * 128 : ci * 128 + csz, lo : half * 1024 + 1024
                    ]
                    if on_pool:
                        nc.gpsimd.dma_start(dst, src)
                    else:
                        nc.sync.dma_start(dst, src)

                return go

            def mk_proj(b, nm, tc_, on_act):
                """qT/kT chunk: stationary W slice, moving xT; PSUM accum over C."""
                off = 0 if nm == "q" else H

                def go():
                    st = state[b]
                    dst = st[nm]
                    pp = psx.tile([128, 512], F32, tag="p", name="pp")[:96, :]
                    for ci in range(NCT):
                        csz = min(128, C - ci * 128)
                        nc.tensor.matmul(
                            pp[:, :],
                            w_sb[:csz, ci, off : off + H],
                            st["xt"][ci][:csz, tc_ * 512 : tc_ * 512 + 512],
                            start=(ci == 0),
                            stop=(ci == NCT - 1),
                        )
                    if nm == "k":
                        # split the copy so the first half unblocks its
                        # score group before the second half drains
                        for h2 in range(2):
                            cdst = dst[:, tc_ * 512 + h2 * 256 : tc_ * 512 + h2 * 256 + 256]
                            csrc = pp[:, h2 * 256 : h2 * 256 + 256]
                            if on_act:
                                nc.scalar.copy(cdst, csrc)
                            else:
                                nc.vector.tensor_copy(cdst, csrc)
                    else:
                        cdst = dst[:, tc_ * 512 : tc_ * 512 + 512]
                        if on_act:
                            nc.scalar.copy(cdst, pp[:, :])
                        else:
                            nc.vector.tensor_copy(cdst, pp[:, :])

                return go

            def mk_vcol(b):
                def go():
                    st = state[b]
                    vp, mf = st["vp"], st["mf"]
                    src = mf[:].rearrange("p (k o) -> p k o", o=1)
                    nc.vector.tensor_copy(vp[:, :, H : H + 1], src)
                    nc.vector.tensor_copy(vp[:, :, H + 1 : H + 2], src)

                return go

            def mk_v(b, tt):
                """v natural tile: stationary xT slice, moving Wv slice."""

                def go():
                    st = state[b]
                    pv = psx.tile([128, 512], F32, tag="p", name="pv")
                    for ci in range(NCT):
                        csz = min(128, C - ci * 128)
                        nc.tensor.matmul(
                            pv[:, :96],
                            st["xt"][ci][:csz, tt * 128 : tt * 128 + 128],
                            w_sb[:csz, ci, 2 * H : 3 * H],
                            start=(ci == 0),
                            stop=(ci == NCT - 1),
                        )
                    nc.vector.tensor_scalar_mul(
                        st["vp"][:, tt, :H], pv[:, :96], st["mf"][:, tt : tt + 1]
                    )

                return go

            def mk_score(b, qc, g):
                def go():
                    st = state[b]
                    ps = pss.tile([128, KG, 512], F32, tag="s", name="ps")
                    st["ps", qc, g] = ps
                    for j in range(KG):
                        kt = g * KG + j
                        nc.tensor.matmul(
                            ps[:, j, :],
                            st["k"][:, kt * 128 : kt * 128 + 128],
                            st["q"][:, qc * 512 : qc * 512 + 512],
                            start=True,
                            stop=True,
                        )

                return go

            def mk_exp(b, qc, g, on_pool=False):
                def go():
                    st = state[b]
                    es = es_pool.tile([128, KG, 512], BF16, tag="es", name="es")
                    st["es", qc, g] = es
                    ps = st.pop(("ps", qc, g))
                    if on_pool:
                        eng = nc.gpsimd
                        bias = nc.const_aps.scalar_like(0.0, ps[:])
                        eng.add_instruction(
                            mybir.InstActivation(
                                name=nc.get_next_instruction_name(),
                                func=exp,
                                ins=[
                                    eng.lower_ap(ps[:]),
                                    eng.lower_ap(bias),
                                    mybir.ImmediateValue(dtype=F32, value=SCALE),
                                    mybir.ImmediateValue(dtype=F32, value=0.0),
                                ],
                                outs=[eng.lower_ap(es[:])],
                            )
                        )
                    else:
                        nc.scalar.activation(es[:], ps[:], exp, scale=SCALE)

                return go

            def chain_part(st, b, qc, jq, lo, hi, split_store=False, alt_pool=False):
                """out-chain piece for one qt-tile: accumulating matmuls kt
                lo..hi-1 into this chain's dedicated PSUM bank (one open
                group per bank at a time); fin + (maybe) store at the end.
                alt_pool borrows the proj/v pool's banks (idle in the final
                blocks) so four chains can be open at once."""
                if ("po", qc, jq) not in st:
                    if alt_pool:
                        st["po", qc, jq] = psx.tile([128, 512], F32, tag="p", name="po")
                    else:
                        st["po", qc, jq] = pso.tile([128, 512], F32, tag="o", name="po")
                po = st["po", qc, jq]
                for kt in range(lo, hi):
                    es = st["es", qc, kt // KG]
                    nc.tensor.matmul(
                        po[:, :HP],
                        es[:, kt % KG, jq * 128 : jq * 128 + 128],
                        st["vp"][:, kt, :],
                        start=(kt == 0),
                        stop=(kt == NT - 1),
                    )
                if hi < NT:
                    return
                st.pop(("po", qc, jq))
                rec = st["rec", qc]
                ot = st["ot", qc]
                nc.vector.reciprocal(rec[:, jq : jq + 1], po[:, H : H + 1])
                if split_store and jq % 2 == 1:
                    nc.scalar.mul(ot[:, jq, :], po[:, :H], rec[:, jq : jq + 1])
                else:
                    nc.vector.tensor_scalar_mul(
                        ot[:, jq, :], po[:, :H], rec[:, jq : jq + 1]
                    )
                if split_store and jq == NQC - 2:
                    # final block: ship qt-tiles 0-2 early so only a small
                    # quarter store sits on the critical tail
                    dst = out_d.ap()[b, qc * 512 : (qc + 1) * 512, :].rearrange(
                        "(p j) h -> p j h", j=NQC
                    )[:, 0 : NQC - 1, :]
                    nc.sync.dma_start(dst, st["ot", qc][:, 0 : NQC - 1, :])
                if jq == NQC - 1:
                    for g in range(NG):
                        st.pop(("es", qc, g))
                    dst = out_d.ap()[b, qc * 512 : (qc + 1) * 512, :].rearrange(
                        "(p j) h -> p j h", j=NQC
                    )
                    ot_t = st.pop(("ot", qc))
                    if split_store:
                        nc.sync.dma_start(
                            dst[:, NQC - 1 : NQC, :], ot_t[:, NQC - 1 : NQC, :]
                        )
                    else:
                        nc.sync.dma_start(dst, ot_t)

            def mk_chain(b, qc, jq, lo=0, hi=NT, split_store=False, alt_pool=False):
                def go():
                    st = state[b]
                    if ("ot", qc) not in st:
                        st["rec", qc] = ot_pool.tile(
                            [128, NQC], F32, tag="rec", name="rec"
                        )
                        st["ot", qc] = ot_pool.tile(
                            [128, NQC, H], F32, tag="ot", name="ot"
                        )
                    chain_part(st, b, qc, jq, lo, hi, split_store, alt_pool)

                return go

            # ---- allocate persistent tiles ---------------------------------
            for b in range(BPC):
                state[b] = {}
                st = state[b]
                st["xt"] = [
                    xt_pool.tile([128, T], BF16, tag=f"xt{ci}", name=f"xt{ci}_{b}")
                    for ci in range(NCT)
                ]
                st["q"] = qk_pool.tile([96, T], BF16, tag="q", name=f"q{b}")
                st["k"] = qk_pool.tile([96, T], BF16, tag="k", name=f"k{b}")
                st["vp"] = vp_pool.tile([128, NT, HP], BF16, name=f"vp{b}")

            def c_phase(blocks, fills, pending=None):
                """blocks: list of (b, qc). Chains of block i ride inside
                block i+1's score/exp stream (slots g=1,3,5,7); the final
                block's chains flush at the end."""
                for bi, (b, qc) in enumerate(blocks):
                    last = bi == len(blocks) - 1
                    mk_score(b, qc, 0)()
                    for g in range(NG):
                        mk_exp(b, qc, g, on_pool=(last and g in POOL_EXP_GROUPS))()
                        if g + 1 < NG:
                            mk_score(b, qc, g + 1)()
                        if pending is not None:
                            if not last and g % 2 == 1:
                                pending[g // 2]()
                            elif last and g <= 3:
                                pending[g]()
                        for u in fills.get((b, qc), [[]] * NG)[g]:
                            u()
                        if last and g == 4:
                            mk_chain(b, qc, 0, 0, NT // 2)()
                        if last and g == 5:
                            mk_chain(b, qc, 1, 0, NT // 2)()
                        if last and g == 6:
                            mk_chain(b, qc, 2, 0, NT // 2, alt_pool=True)()
                            mk_chain(b, qc, 0, NT // 2, NT - KG)()
                        if last and g == 7:
                            mk_chain(b, qc, 3, 0, NT // 2, alt_pool=True)()
                            mk_chain(b, qc, 1, NT // 2, NT - KG)()
                            mk_chain(b, qc, 2, NT // 2, NT - KG, alt_pool=True)()
                            mk_chain(b, qc, 3, NT // 2, NT - KG, alt_pool=True)()
                    if last:
                        mk_chain(b, qc, 0, NT - KG, NT, split_store=True)()
                        mk_chain(b, qc, 1, NT - KG, NT, split_store=True)()
                        mk_chain(b, qc, 2, NT - KG, NT, split_store=True, alt_pool=True)()
                        mk_chain(b, qc, 3, NT - KG, NT, split_store=True, alt_pool=True)()
                    else:
                        pending = [mk_chain(b, qc, jq) for jq in range(NQC)]

            def run(units):
                for u in units:
                    u()

            def P(b, nm, tc_, on_act=False):
                return mk_proj(b, nm, tc_, on_act)

            def V(b, *tts):
                return [mk_v(b, tt) for tt in tts]

            # ---- emission ---------------------------------------------------
            # DMA order: W first (needed by every matmul), then b0 x half 0
            # (kT0/qT0 critical path), mask 0, b0 x half 1, mask 1.
            # b0 x on SP/HWDGE, b1 x on GpSimd/SWDGE (parallel desc-gen).
            for ci in range(NCT):
                mk_xdma(0, 0, ci, on_pool=False)()
            mk_mask(0)()
            for ci in range(NCT):
                mk_xdma(0, 1, ci, on_pool=False)()
            mk_mask(1)()
            for half in range(2):
                for ci in range(NCT):
                    mk_xdma(1, half, ci, on_pool=False)()

            # b0 head: kT0 and qT0 matmuls interleaved per c-tile so both
            # PSUM groups fill as x arrives; copies land on ACT (k) and DVE
            # (q) in parallel. Then mask col + first v tiles for out(qc0,0).
            ppk = psx.tile([128, 512], F32, tag="p", name="ppk")[:96, :]
            ppq = psx.tile([128, 512], F32, tag="p", name="ppq")[:96, :]
            st0 = state[0]
            for ci in range(NCT):
                csz = min(128, C - ci * 128)
                for pp, off in ((ppk, H), (ppq, 0)):
                    nc.tensor.matmul(
                        pp[:, :],
                        w_sb[:csz, ci, off : off + H],
                        st0["xt"][ci][:csz, 0:512],
                        start=(ci == 0),
                        stop=(ci == NCT - 1),
                    )
            nc.scalar.copy(st0["k"][:, 0:512], ppk[:, :])
            nc.vector.tensor_copy(st0["q"][:, 0:512], ppq[:, :])
            run([mk_vcol(0)] + V(0, 0, 1))

            # Fill ledger: score(b,qc,g) needs kT chunk g//2 and qT chunk
            # qc; chains(b,qc) need all 16 v tiles of batch b.
            #
            # The first two blocks are emitted as four HALF-blocks
            # (qc0-lo, qc1-lo, qc0-hi, qc1-hi): the lo halves only touch
            # x half 0, giving ACT an 8-group runway while x half 1 (which
            # gates kT chunks 2-3) is still in flight.
            def seg(b, qc, glo, ghi, segfills):
                mk_score(b, qc, glo)()
                for i, g in enumerate(range(glo, ghi)):
                    mk_exp(b, qc, g)()
                    if g + 1 < ghi:
                        mk_score(b, qc, g + 1)()
                    for u in segfills[i]:
                        u()

            seg(0, 0, 0, 4, [[P(0, "k", 1)], V(0, 2, 3), [P(0, "q", 1)], V(0, 4, 5)])
            seg(0, 1, 0, 4, [[P(0, "k", 2)], V(0, 6, 7), [P(0, "k", 3)], V(0, 8, 9)])
            seg(0, 0, 4, 8, [[P(0, "q", 2)], V(0, 10, 11), V(0, 12, 13), V(0, 14, 15)])
            seg(
                0,
                1,
                4,
                8,
                [
                    [mk_chain(0, 0, 0)],
                    [mk_chain(0, 0, 1), P(0, "q", 3)],
                    [mk_chain(0, 0, 2)],
                    [mk_chain(0, 0, 3)],
                ],
            )

            E = []
            fills = {
                (0, 2): [
                    [P(1, "k", 0)],
                    [mk_vcol(1)],
                    [P(1, "q", 0)],
                    E,
                    [P(1, "k", 1)],
                    E,
                    E,
                    E,
                ],
                (0, 3): [
                    [P(1, "k", 2)],
                    V(1, 0, 1),
                    E,
                    V(1, 2, 3),
                    E,
                    V(1, 4, 5),
                    E,
                    E,
                ],
                (1, 0): [
                    [P(1, "k", 3)],
                    V(1, 6, 7),
                    V(1, 8, 9),
                    V(1, 10, 11),
                    E,
                    [P(1, "q", 1)],
                    E,
                    E,
                ],
                (1, 1): [
                    V(1, 12, 13) + V(1, 14, 15),
                    [P(1, "q", 2)],
                    E,
                    E,
                    E,
                    E,
                    E,
                    E,
                ],
                (1, 2): [[P(1, "q", 3)], E, E, E, E, E, E, E],
                (1, 3): [E] * NG,
            }
            blocks = [(0, 2), (0, 3)] + [(1, qc) for qc in range(NQC)]
            c_phase(blocks, fills, pending=[mk_chain(0, 1, jq) for jq in range(NQC)])

    _split_excess_waits(nc)
    return nc


_prog = None


def _get_prog():
    global _prog
    if _prog is None:
        _prog = _build()
    return _prog


def kernel(x, mask, Wk, Wq, Wv, **_ignored):
    bf16 = ml_dtypes.bfloat16
    # host-side prep: transpose x, cast to bf16, pack weights (zero-pad C)
    xt = np.ascontiguousarray(
        np.asarray(x, dtype=np.float32).transpose(0, 2, 1)
    ).astype(bf16)
    w = np.zeros((NCT * 128, 3 * H), dtype=bf16)
    w[:C] = np.concatenate(
        [np.asarray(Wq), np.asarray(Wk), np.asarray(Wv)], axis=1
    ).astype(bf16)
    maskf = np.asarray(mask).astype(np.float32).reshape(B, NT, 128).transpose(0, 2, 1)
    maskf = np.ascontiguousarray(maskf)

    nc = _get_prog()
    in_maps = [
        {
            "xt": xt[i * BPC : (i + 1) * BPC],
            "w": w,
            "maskf": maskf[i * BPC : (i + 1) * BPC],
        }
        for i in range(N_CORES)
    ]
    res = run_bass_kernel_spmd(nc, in_maps, core_ids=list(range(N_CORES)))
    raw = np.concatenate([res.results[i]["out"] for i in range(N_CORES)], axis=0)
    # un-permute the store layout: dram row qc*512 + 4p+j holds q = qc*512+128j+p
    out = raw.reshape(B, NQC, 128, 4, H).transpose(0, 1, 3, 2, 4).reshape(B, T, H)
    return np.ascontiguousarray(out)


if __name__ == "__main__":
    rng = np.random.default_rng(0)
    x = rng.standard_normal((B, T, C), dtype=np.float32)
    mask = np.ones((B, T), dtype=bool)
    s = 1.0 / np.sqrt(C)
    Wk = (rng.standard_normal((C, H)) * s).astype(np.float32)
    Wq = (rng.standard_normal((C, H)) * s).astype(np.float32)
    Wv = (rng.standard_normal((C, H)) * s).astype(np.float32)
    out = kernel(x, mask=mask, Wk=Wk, Wq=Wq, Wv=Wv)
    print("out", out.shape, out.dtype, float(np.abs(out).max()))


# revision 59
# speedup vs baseline: 1.0351x; 1.0086x over previous
"""Single-head attention (B=16, T=2048, C=576, H=96) on 8 TRN2 NeuronCores.

Sharding: data-parallel over batch — 2 batches per core; weights replicated.

All matmul operands are bf16 (fp32 PSUM accumulation); rel-err budget is
2e-2 and bf16 end-to-end measures ~7e-3. The host pre-transposes x to
xT=[C,T] bf16 and packs W=[5,128,288] (zero-padded C) bf16, so the device
never transposes anything:

  qT,kT [96,T]  = W-slice stationary, xT moving          (PSUM-accum over C)
  v_nat [T,96]  = xT-tile stationary, Wv-slice moving    (natural layout)
  vp [128,NT,98]: v*mask with mask in cols 96,97 (denominator trick)
  sT [kpos,qpos] per (qc, 2-kt group) into a 2-bank PSUM tile (bufs=2) so
     the next group's score matmuls overlap the previous group's exp
  out_nat [qpos,98] = es-slice stationary, vp moving, accumulated over kt;
     col 96 is the softmax denominator. DVE reciprocal+scale, DMA out with
     a (p j) permutation for 1536B descriptors; host un-permutes.

All x loads ride the SP/HWDGE queue in priority order (b0 half 0 first);
the tiny mask loads go through the idle GpSimd engine's SWDGE so they do
not consume SP issue slots.

This walrus build rejects >1 sync wait per instruction (and any wait on a
Drain), so after TileContext builds the module we hoist excess waits onto
injected same-engine NOPs — semantics identical since engines execute
their stream in order.
"""

import sys

if "/opt/trn_rl_repo" not in sys.path:
    sys.path.insert(0, "/opt/trn_rl_repo")

import ml_dtypes
import numpy as np

import concourse.bass as bass
import concourse.tile as tile
from concourse import mybir
from concourse.bass_utils import run_bass_kernel_spmd

N_CORES = 8
B, T, C, H = 16, 2048, 576, 96
BPC = B // N_CORES  # batches per core
SCALE = 1.0 / float(np.sqrt(H))

F32 = mybir.dt.float32
BF16 = mybir.dt.bfloat16

NT = T // 128  # 16 key tiles
NCT = (C + 127) // 128  # 5 c-tiles (last is 64)
NQC = T // 512  # 4 query chunks
NG = 8  # kt-groups per query chunk (2 kt each)
KG = NT // NG  # 2 kt per group
HP = H + 2  # 98: H + denominator col + dup (even moving count)
POOL_EXP_GROUPS = ()  # which kt-groups' exp runs on GpSimd instead of ACT


def _split_excess_waits(nc, max_waits=1):
    """Hoist sync waits beyond this walrus's per-instruction limit onto
    injected NOPs that run just before, on the same engine."""
    n_split = 0
    for fn in nc.m.functions:
        for blk in fn.blocks:
            new_insts = []
            changed = False
            for inst in blk.instructions:
                si = inst.sync_info
                waits = list(si.on_wait) if si is not None else []
                cap = 0 if isinstance(inst, mybir.InstDrain) else max_waits
                if len(waits) > cap:
                    # Keep the most meaningful wait ON the instruction (its
                    # engine-stage wait doesn't block the sequencer); push
                    # self-engine sems (trivially satisfied in-order) and
                    # DMA-completion WARs onto the NOPs, which DO block SEQ.
                    eng = str(inst.engine).split(".")[-1].split(":")[0].strip("'\" >")

                    def prio(iw):
                        i, w = iw
                        nm = getattr(w, "ant_name", "") or ""
                        self_sem = nm.startswith(eng)
                        dma_sem = nm.startswith("DMAHW") or nm.startswith("DMASW")
                        return (0 if self_sem else (1 if dma_sem else 2), i)

                    order = sorted(enumerate(waits), key=prio)
                    waits = [w for _, w in order]
                    excess = waits[:-cap] if cap else waits
                    keep = waits[-cap:] if cap else []
                    for i in range(0, len(excess), max_waits):
                        chunk = excess[i : i + max_waits]
                        new_insts.append(
                            mybir.InstNoOp(
                                name=f"{inst.name}-wsplit{i}",
                                engine=inst.engine,
                                ins=[],
                                outs=[],
                                sync_info=mybir.SyncInfo(on_wait=chunk, on_update=[]),
                            )
                        )
                    inst.sync_info = mybir.SyncInfo(
                        on_wait=keep, on_update=list(si.on_update)
                    )
                    changed = True
                    n_split += 1
                new_insts.append(inst)
            if changed:
                blk.instructions = new_insts
    return n_split


def _build():
    nc = bass.Bass("TRN2", target_bir_lowering=False, debug=False)

    xt_d = nc.dram_tensor("xt", [BPC, C, T], BF16, kind="ExternalInput")
    w_d = nc.dram_tensor("w", [NCT * 128, 3 * H], BF16, kind="ExternalInput")
    mf_d = nc.dram_tensor("maskf", [BPC, 128, NT], F32, kind="ExternalInput")
    out_d = nc.dram_tensor("out", [BPC, T, H], F32, kind="ExternalOutput")

    exp = mybir.ActivationFunctionType.Exp

    with tile.TileContext(nc) as tc:
        with (
            tc.tile_pool(name="const", bufs=1) as const_pool,
            tc.tile_pool(name="xt", bufs=2) as xt_pool,
            tc.tile_pool(name="qk", bufs=2) as qk_pool,
            tc.tile_pool(name="vp", bufs=2) as vp_pool,
            tc.tile_pool(name="mk", bufs=2) as mk_pool,
            tc.tile_pool(name="es", bufs=18) as es_pool,
            tc.tile_pool(name="ot", bufs=4) as ot_pool,
            tc.tile_pool(name="psx", bufs=2, space="PSUM") as psx,  # proj+v
            tc.tile_pool(name="pso", bufs=2, space="PSUM") as pso,  # out chains
            tc.tile_pool(name="pss", bufs=2, space="PSUM") as pss,  # scores 2-bank
        ):
            # PE p-state warm-up: dependency-free dummy matmuls on garbage
            # SBUF data while the head DMAs land (PE is idle anyway). After
            # ~3us of continuous busy the cost model (and hardware) runs the
            # array at full clock, so the first real matmuls aren't 2x slow.
            # The psum bank is reset by its next user's start=True matmul.
            dmy = const_pool.tile([128, 640], BF16, name="dmy")
            nc.vector.memset(dmy[:], 0)
            dpo = pso.tile([128, 512], F32, tag="o", name="dpo")
            for _ in range(1):
                nc.tensor.matmul(
                    dpo[:, :],
                    dmy[:, 0:128],
                    dmy[:, 128:640],
                    start=True,
                    stop=True,
                )

            w_sb = const_pool.tile([128, NCT, 3 * H], BF16, name="w_sb")
            nc.sync.dma_start(
                w_sb[:], w_d.ap().rearrange("(g p) c -> p g c", p=128)
            )
            # pre-warm the exp table so the first real exp doesn't pay the
            # table load inside the pipeline
            warm = const_pool.tile([128, 2], F32, name="warm")
            nc.scalar.activation(warm[:], w_sb[:, 0, 0:4].bitcast(F32), exp)

            state = {}

            def mk_mask(b):
                def go():
                    mf = mk_pool.tile([128, NT], F32, name=f"mf{b}")
                    state[b]["mf"] = mf
                    nc.gpsimd.dma_start(mf[:], mf_d.ap()[b])

                return go

            def mk_xdma(b, lo, hi, ci):
                def go():
                    st = state[b]
                    csz = min(128, C - ci * 128)
                    nc.sync.dma_start(
                        st["xt"][ci][:csz, lo:hi],
                        xt_d.ap()[b, ci * 128 : ci * 128 + csz, lo:hi],
                    )

                return go

            def mk_proj(b, nm, tc_, on_act):
                """qT/kT chunk: stationary W slice, moving xT; PSUM accum over C."""
                off = 0 if nm == "q" else H

                def go():
                    st = state[b]
                    dst = st[nm]
                    pp = psx.tile([128, 512], F32, tag="p", name="pp")[:96, :]
                    for ci in range(NCT):
                        csz = min(128, C - ci * 128)
                        nc.tensor.matmul(
                            pp[:, :],
                            w_sb[:csz, ci, off : off + H],
                            st["xt"][ci][:csz, tc_ * 512 : tc_ * 512 + 512],
                            start=(ci == 0),
                            stop=(ci == NCT - 1),
                        )
                    if nm == "k":
                        # split the copy so the first half unblocks its
                        # score group before the second half drains
                        for h2 in range(2):
                            cdst = dst[:, tc_ * 512 + h2 * 256 : tc_ * 512 + h2 * 256 + 256]
                            csrc = pp[:, h2 * 256 : h2 * 256 + 256]
                            if on_act:
                                nc.scalar.copy(cdst, csrc)
                            else:
                                nc.vector.tensor_copy(cdst, csrc)
                    else:
                        cdst = dst[:, tc_ * 512 : tc_ * 512 + 512]
                        if on_act:
                            nc.scalar.copy(cdst, pp[:, :])
                        else:
                            nc.vector.tensor_copy(cdst, pp[:, :])

                return go

            def mk_vcol(b):
                def go():
                    st = state[b]
                    vp, mf = st["vp"], st["mf"]
                    src = mf[:].rearrange("p (k o) -> p k o", o=1)
                    nc.vector.tensor_copy(vp[:, :, H : H + 1], src)
                    nc.vector.tensor_copy(vp[:, :, H + 1 : H + 2], src)

                return go

            def mk_v(b, tt):
                """v natural tile: stationary xT slice, moving Wv slice."""

                def go():
                    st = state[b]
                    pv = psx.tile([128, 512], F32, tag="p", name="pv")
                    for ci in range(NCT):
                        csz = min(128, C - ci * 128)
                        nc.tensor.matmul(
                            pv[:, :96],
                            st["xt"][ci][:csz, tt * 128 : tt * 128 + 128],
                            w_sb[:csz, ci, 2 * H : 3 * H],
                            start=(ci == 0),
                            stop=(ci == NCT - 1),
                        )
                    nc.vector.tensor_scalar_mul(
                        st["vp"][:, tt, :H], pv[:, :96], st["mf"][:, tt : tt + 1]
                    )

                return go

            def mk_score(b, qc, g):
                def go():
                    st = state[b]
                    ps = pss.tile([128, KG, 512], F32, tag="s", name="ps")
                    st["ps", qc, g] = ps
                    for j in range(KG):
                        kt = g * KG + j
                        nc.tensor.matmul(
                            ps[:, j, :],
                            st["k"][:, kt * 128 : kt * 128 + 128],
                            st["q"][:, qc * 512 : qc * 512 + 512],
                            start=True,
                            stop=True,
                        )

                return go

            def mk_exp(b, qc, g, on_pool=False):
                def go():
                    st = state[b]
                    es = es_pool.tile([128, KG, 512], BF16, tag="es", name="es")
                    st["es", qc, g] = es
                    ps = st.pop(("ps", qc, g))
                    if on_pool:
                        eng = nc.gpsimd
                        bias = nc.const_aps.scalar_like(0.0, ps[:])
                        eng.add_instruction(
                            mybir.InstActivation(
                                name=nc.get_next_instruction_name(),
                                func=exp,
                                ins=[
                                    eng.lower_ap(ps[:]),
                                    eng.lower_ap(bias),
                                    mybir.ImmediateValue(dtype=F32, value=SCALE),
                                    mybir.ImmediateValue(dtype=F32, value=0.0),
                                ],
                                outs=[eng.lower_ap(es[:])],
                            )
                        )
                    else:
                        nc.scalar.activation(es[:], ps[:], exp, scale=SCALE)

                return go

            def chain_part(st, b, qc, jq, lo, hi, split_store=False, alt_pool=False):
                """out-chain piece for one qt-tile: accumulating matmuls kt
                lo..hi-1 into this chain's dedicated PSUM bank (one open
                group per bank at a time); fin + (maybe) store at the end.
                alt_pool borrows the proj/v pool's banks (idle in the final
                blocks) so four chains can be open at once."""
                if ("po", qc, jq) not in st:
                    if alt_pool:
                        st["po", qc, jq] = psx.tile([128, 512], F32, tag="p", name="po")
                    else:
                        st["po", qc, jq] = pso.tile([128, 512], F32, tag="o", name="po")
                po = st["po", qc, jq]
                for kt in range(lo, hi):
                    es = st["es", qc, kt // KG]
                    nc.tensor.matmul(
                        po[:, :HP],
                        es[:, kt % KG, jq * 128 : jq * 128 + 128],
                        st["vp"][:, kt, :],
                        start=(kt == 0),
                        stop=(kt == NT - 1),
                    )
                if hi < NT:
                    return
                st.pop(("po", qc, jq))
                rec = st["rec", qc]
                ot = st["ot", qc]
                nc.vector.reciprocal(rec[:, jq : jq + 1], po[:, H : H + 1])
                if split_store and jq % 2 == 1:
                    nc.scalar.mul(ot[:, jq, :], po[:, :H], rec[:, jq : jq + 1])
                else:
                    nc.vector.tensor_scalar_mul(
                        ot[:, jq, :], po[:, :H], rec[:, jq : jq + 1]
                    )
                if split_store and jq == NQC - 2:
                    # final block: ship qt-tiles 0-2 early so only a small
                    # quarter store sits on the critical tail
                    dst = out_d.ap()[b, qc * 512 : (qc + 1) * 512, :].rearrange(
                        "(p j) h -> p j h", j=NQC
                    )[:, 0 : NQC - 1, :]
                    nc.sync.dma_start(dst, st["ot", qc][:, 0 : NQC - 1, :])
                if jq == NQC - 1:
                    for g in range(NG):
                        st.pop(("es", qc, g))
                    dst = out_d.ap()[b, qc * 512 : (qc + 1) * 512, :].rearrange(
                        "(p j) h -> p j h", j=NQC
                    )
                    ot_t = st.pop(("ot", qc))
                    if split_store:
                        nc.sync.dma_start(
                            dst[:, NQC - 1 : NQC, :], ot_t[:, NQC - 1 : NQC, :]
                        )
                    else:
                        nc.sync.dma_start(dst, ot_t)

            def mk_chain(b, qc, jq, lo=0, hi=NT, split_store=False, alt_pool=False):
                def go():
                    st = state[b]
                    if ("ot", qc) not in st:
                        st["rec", qc] = ot_pool.tile(
                            [128, NQC], F32, tag="rec", name="rec"
                        )
                        st["ot", qc] = ot_pool.tile(
                            [128, NQC, H], F32, tag="ot", name="ot"
                        )
                    chain_part(st, b, qc, jq, lo, hi, split_store, alt_pool)

                return go

            # ---- allocate persistent tiles ---------------------------------
            for b in range(BPC):
                state[b] = {}
                st = state[b]
                st["xt"] = [
                    xt_pool.tile([128, T], BF16, tag=f"xt{ci}", name=f"xt{ci}_{b}")
                    for ci in range(NCT)
                ]
                st["q"] = qk_pool.tile([96, T], BF16, tag="q", name=f"q{b}")
                st["k"] = qk_pool.tile([96, T], BF16, tag="k", name=f"k{b}")
                st["vp"] = vp_pool.tile([128, NT, HP], BF16, name=f"vp{b}")

            def c_phase(blocks, fills, pending=None):
                """blocks: list of (b, qc). Chains of block i ride inside
                block i+1's score/exp stream (slots g=1,3,5,7); the final
                block's chains flush at the end."""
                for bi, (b, qc) in enumerate(blocks):
                    last = bi == len(blocks) - 1
                    mk_score(b, qc, 0)()
                    for g in range(NG):
                        mk_exp(b, qc, g, on_pool=(last and g in POOL_EXP_GROUPS))()
                        if g + 1 < NG:
                            mk_score(b, qc, g + 1)()
                        if pending is not None:
                            if not last and g % 2 == 1:
                                pending[g // 2]()
                            elif last and g <= 3:
                                pending[g]()
                        for u in fills.get((b, qc), [[]] * NG)[g]:
                            u()
                        if last and g == 4:
                            mk_chain(b, qc, 0, 0, NT // 2)()
                        if last and g == 5:
                            mk_chain(b, qc, 1, 0, NT // 2)()
                        if last and g == 6:
                            mk_chain(b, qc, 2, 0, NT // 2, alt_pool=True)()
                            mk_chain(b, qc, 0, NT // 2, NT - KG)()
                        if last and g == 7:
                            mk_chain(b, qc, 3, 0, NT // 2, alt_pool=True)()
                            mk_chain(b, qc, 1, NT // 2, NT - KG)()
                            mk_chain(b, qc, 2, NT // 2, NT - KG, alt_pool=True)()
                            mk_chain(b, qc, 3, NT // 2, NT - KG, alt_pool=True)()
                    if last:
                        mk_chain(b, qc, 0, NT - KG, NT, split_store=True)()
                        mk_chain(b, qc, 1, NT - KG, NT, split_store=True)()
                        mk_chain(b, qc, 2, NT - KG, NT, split_store=True, alt_pool=True)()
                        mk_chain(b, qc, 3, NT - KG, NT, split_store=True, alt_pool=True)()
                    else:
                        pending = [mk_chain(b, qc, jq) for jq in range(NQC)]

            def run(units):
                for u in units:
                    u()

            def P(b, nm, tc_, on_act=False):
                return mk_proj(b, nm, tc_, on_act)

            def V(b, *tts):
                return [mk_v(b, tt) for tt in tts]

            # ---- emission ---------------------------------------------------
            # DMA order: W first (needed by every matmul), then b0 x half 0
            # (kT0/qT0 critical path), mask 0, b0 x half 1, mask 1.
            # b0 x on SP/HWDGE, b1 x on GpSimd/SWDGE (parallel desc-gen).
            # b0: chunk 0 first (all kT0/qT0 needs), then chunk 1, then
            # half 1 — the half-block prefix relaxes the chunk-2/3 deadline
            # so the finer early pieces start the exp stream sooner.
            for ci in range(NCT):
                mk_xdma(0, 0, 512, ci)()
            mk_mask(0)()
            for ci in range(NCT):
                mk_xdma(0, 512, 1024, ci)()
            for ci in range(NCT):
                mk_xdma(0, 1024, 2048, ci)()
            mk_mask(1)()
            for half in range(2):
                for ci in range(NCT):
                    mk_xdma(1, half * 1024, half * 1024 + 1024, ci)()

            # b0 head: kT0 and qT0 matmuls interleaved per c-tile so both
            # PSUM groups fill as x arrives; copies land on ACT (k) and DVE
            # (q) in parallel. Then mask col + first v tiles for out(qc0,0).
            ppk = psx.tile([128, 512], F32, tag="p", name="ppk")[:96, :]
            ppq = psx.tile([128, 512], F32, tag="p", name="ppq")[:96, :]
            st0 = state[0]
            for ci in range(NCT):
                csz = min(128, C - ci * 128)
                for pp, off in ((ppk, H), (ppq, 0)):
                    nc.tensor.matmul(
                        pp[:, :],
                        w_sb[:csz, ci, off : off + H],
                        st0["xt"][ci][:csz, 0:512],
                        start=(ci == 0),
                        stop=(ci == NCT - 1),
                    )
            nc.scalar.copy(st0["k"][:, 0:512], ppk[:, :])
            nc.vector.tensor_copy(st0["q"][:, 0:512], ppq[:, :])
            run([mk_vcol(0)] + V(0, 0, 1))

            # Fill ledger: score(b,qc,g) needs kT chunk g//2 and qT chunk
            # qc; chains(b,qc) need all 16 v tiles of batch b.
            #
            # The first two blocks are emitted as four HALF-blocks
            # (qc0-lo, qc1-lo, qc0-hi, qc1-hi): the lo halves only touch
            # x half 0, giving ACT an 8-group runway while x half 1 (which
            # gates kT chunks 2-3) is still in flight.
            def seg(b, qc, glo, ghi, segfills):
                mk_score(b, qc, glo)()
                for i, g in enumerate(range(glo, ghi)):
                    mk_exp(b, qc, g)()
                    if g + 1 < ghi:
                        mk_score(b, qc, g + 1)()
                    for u in segfills[i]:
                        u()

            seg(0, 0, 0, 4, [[P(0, "k", 1)], V(0, 2, 3), [P(0, "q", 1)], V(0, 4, 5)])
            seg(0, 1, 0, 4, [[P(0, "k", 2)], V(0, 6, 7), [P(0, "k", 3)], V(0, 8, 9)])
            seg(0, 0, 4, 8, [[P(0, "q", 2)], V(0, 10, 11), V(0, 12, 13), V(0, 14, 15)])
            seg(
                0,
                1,
                4,
                8,
                [
                    [mk_chain(0, 0, 0)],
                    [mk_chain(0, 0, 1), P(0, "q", 3)],
                    [mk_chain(0, 0, 2)],
                    [mk_chain(0, 0, 3)],
                ],
            )

            E = []
            fills = {
                (0, 2): [
                    [P(1, "k", 0)],
                    [mk_vcol(1)],
                    [P(1, "q", 0)],
                    E,
                    [P(1, "k", 1)],
                    E,
                    E,
                    E,
                ],
                (0, 3): [
                    [P(1, "k", 2)],
                    V(1, 0, 1),
                    E,
                    V(1, 2, 3),
                    E,
                    V(1, 4, 5),
                    E,
                    E,
                ],
                (1, 0): [
                    [P(1, "k", 3)],
                    V(1, 6, 7),
                    V(1, 8, 9),
                    V(1, 10, 11),
                    E,
                    [P(1, "q", 1)],
                    E,
                    E,
                ],
                (1, 1): [
                    V(1, 12, 13) + V(1, 14, 15),
                    [P(1, "q", 2)],
                    E,
                    E,
                    E,
                    E,
                    E,
                    E,
                ],
                (1, 2): [[P(1, "q", 3)], E, E, E, E, E, E, E],
                (1, 3): [E] * NG,
            }
            blocks = [(0, 2), (0, 3)] + [(1, qc) for qc in range(NQC)]
            c_phase(blocks, fills, pending=[mk_chain(0, 1, jq) for jq in range(NQC)])

    _split_excess_waits(nc)
    return nc


_prog = None


def _get_prog():
    global _prog
    if _prog is None:
        _prog = _build()
    return _prog


def kernel(x, mask, Wk, Wq, Wv, **_ignored):
    bf16 = ml_dtypes.bfloat16
    # host-side prep: transpose x, cast to bf16, pack weights (zero-pad C)
    xt = np.ascontiguousarray(
        np.asarray(x, dtype=np.float32).transpose(0, 2, 1)
    ).astype(bf16)
    w = np.zeros((NCT * 128, 3 * H), dtype=bf16)
    w[:C] = np.concatenate(
        [np.asarray(Wq), np.asarray(Wk), np.asarray(Wv)], axis=1
    ).astype(bf16)
    maskf = np.asarray(mask).astype(np.float32).reshape(B, NT, 128).transpose(0, 2, 1)
    maskf = np.ascontiguousarray(maskf)

    nc = _get_prog()
    in_maps = [
        {
            "xt": xt[i * BPC : (i + 1) * BPC],
            "w": w,
            "maskf": maskf[i * BPC : (i + 1) * BPC],
        }
        for i in range(N_CORES)
    ]
    res = run_bass_kernel_spmd(nc, in_maps, core_ids=list(range(N_CORES)))
    raw = np.concatenate([res.results[i]["out"] for i in range(N_CORES)], axis=0)
    # un-permute the store layout: dram row qc*512 + 4p+j holds q = qc*512+128j+p
    out = raw.reshape(B, NQC, 128, 4, H).transpose(0, 1, 3, 2, 4).reshape(B, T, H)
    return np.ascontiguousarray(out)


if __name__ == "__main__":
    rng = np.random.default_rng(0)
    x = rng.standard_normal((B, T, C), dtype=np.float32)
    mask = np.ones((B, T), dtype=bool)
    s = 1.0 / np.sqrt(C)
    Wk = (rng.standard_normal((C, H)) * s).astype(np.float32)
    Wq = (rng.standard_normal((C, H)) * s).astype(np.float32)
    Wv = (rng.standard_normal((C, H)) * s).astype(np.float32)
    out = kernel(x, mask=mask, Wk=Wk, Wq=Wq, Wv=Wv)
    print("out", out.shape, out.dtype, float(np.abs(out).max()))
